# revision 53
# baseline (speedup 1.0000x reference)
"""Trainium2 Bass kernel for nn_BilinearUpsampler (cell-sorted v3).

out[b,c,i,j] = sum_k softmax_k(MLP(poseMap)[c,k,i,j]) * xpad[b,c,Y[i,j]+dy_k,X[i,j]+dx_k]

Key idea vs the per-pixel-gather baseline: output pixels are grouped by
their source cell (Y,X).  A tile = 128 cells (on partitions) x J slots
(pixels of that cell, on the free axis).  The 3x3 input window (1152
bf16 = one "cell window") is gathered ONCE per cell (128 descriptors
per tile instead of one per pixel -> 16x less DMA) and broadcast over
the cell's pixels with a stride-0 AP in the tap multiply.  Cells are
occupancy-sorted into tiles so J per tile ~= every cell's count (pad
waste ~2-4%).  The schedule is data-dependent; the program is compiled
per schedule signature and cached.

Work placement (tuned against the TimelineSim cost model; PSUM reads
are only legal on PE/ACT/DVE, so all PSUM evictions sit on ACT):
  PE   : MLP (3->64->256) + logits (256->576)
  ACT  : h1/h2 relu evictions, exp eviction (PSUM->SBUF)
  DVE  : tap muls (bf16 2x, g broadcast over j via stride-0 AP), den
         d1/d2 (in-place on e), tree finish (k0+=k1, k0+=k8 in prods),
         reciprocal, final normalize multiply
  Pool : den d3/final into a separate den tile (frees e early -- this
         breaks the exp->...->div buffer cycle that serialized ACT
         behind DVE), SWDGE descriptor-gen for gathers and accum DMAs
  DMA  : per-cell window gather, accumulating SBUF->SBUF tap-tree
         levels L1 (k[0:4]+=k[4:8]) and L2 (k[0:2]+=k[2:4]), pose in,
         out store

Pipeline per iteration i (in-order engine queues head-block on the
first not-ready instruction, so per-engine emission order is chosen to
match dependency readiness): d2+den(i-1) -> logits/exp(i) subtiles with
tap-mul groups spliced in and MLP(i+1) q-blocks in the exp-wait gaps ->
L2(i-1) -> tree-finish+recip+normalize+store(i-1) -> L1(i).
"""

import sys
import os

sys.path.insert(0, "/opt/trn_rl_repo")

import numpy as np
import ml_dtypes

import concourse.bass as bass
import concourse.bacc as bacc
import concourse.mybir as mybir
import concourse.tile as tile
import concourse.alu_op_type as alu
from concourse.bass_utils import run_bass_kernel_spmd

BF16 = mybir.dt.bfloat16
F32 = mybir.dt.float32
I16 = mybir.dt.int16
AF = mybir.ActivationFunctionType
ADD = alu.AluOpType.add

NCORES = 8
C = 64
KS = 3
BS = 2
HI = WI = 128
HO = WO = 512
SCW = HI + 2  # supercell grid width (padded x: 0..129)
NSC = HI * SCW  # supercells
NWIN = NSC - 2  # gatherable 3-supercell windows
PXTOT = HO * WO

JCAP = 16  # max j-slots per chunk

# work-placement knobs (swept against the cost model)
CFG = {
    "d1": "dve",      # den level 1: 'dve' | 'dma'
    "d3fin": "pool",   # den level 3 + final: 'dve' | 'pool'
    "h2relu": "act",  # h2 relu eviction: 'act' (pool cannot read PSUM)
    "poolj": 0,       # j-slots per chunk whose tap-muls run on Pool
    "mgrp": 8,        # j-slots per tap-mul group
}

LAST_RESULT = None
LAST_PROG = None

_PROG_CACHE = {}

LABEL_HOOK = None


def _lbl(s):
    if LABEL_HOOK is not None:
        LABEL_HOOK(s)


def _chunks(J):
    nch = (J + JCAP - 1) // JCAP
    lo = J // nch
    hi = lo + 1
    nhi = J - lo * nch
    return [hi] * nhi + [lo] * (nch - nhi)


def build_program(Js):
    """Js: per-tile slot counts (shared across all 8 cores)."""
    Js = list(Js)
    nt = len(Js)
    ST = sum(Js)
    base = np.concatenate([[0], np.cumsum(Js)])[:-1]
    JMAXT = max(Js)

    nc = bacc.Bacc("TRN2", target_bir_lowering=False, debug=False,
                   num_devices=NCORES)

    xw_d = nc.dram_tensor("xw", [NSC * 384], BF16, kind="ExternalInput")
    idx_d = nc.dram_tensor("idx", [128, nt * 8], I16, kind="ExternalInput")
    pose_d = nc.dram_tensor("pose", [4, ST * 128], BF16, kind="ExternalInput")
    # cpack: [0:576] w3km0, [576:1152] w3km1, [1152:1408] w2t (rows 0-64),
    # [1408:1473] w1t (rows 0-3)
    cpack_d = nc.dram_tensor("cpack", [128, 1473], BF16, kind="ExternalInput")
    out_d = nc.dram_tensor("out", [128, ST, 128], BF16, kind="ExternalOutput")

    import bass_rust

    def x_windows_ap():
        ap = xw_d[:].copy()
        ap.ap = bass_rust.VecI64Pair([(384, NWIN), (1, 1152)])
        return ap

    # flat chunk list: (tile, jb, jc, slot0)
    chunks = []
    for t in range(nt):
        jb = 0
        for jc in _chunks(Js[t]):
            chunks.append((t, jb, jc, int(base[t]) + jb))
            jb += jc
    nch = len(chunks)

    with tile.TileContext(nc) as tc:
        with (
            tc.tile_pool(name="consts", bufs=1) as cpool,
            tc.tile_pool(name="pose", bufs=3) as ppool,
            tc.tile_pool(name="gath", bufs=3) as gpool,
            tc.tile_pool(name="mlp", bufs=2) as mpool,
            tc.tile_pool(name="ework", bufs=2) as epool,
            tc.tile_pool(name="prod", bufs=2) as vpool,
            tc.tile_pool(name="dpool", bufs=2) as dpool,
            tc.tile_pool(name="outp", bufs=2) as opool,
            tc.tile_pool(name="ph1", bufs=1 if CFG["expmerge"] else 2,
                         space="PSUM") as ph1,
            tc.tile_pool(name="ph2", bufs=1, space="PSUM") as ph2,
            tc.tile_pool(name="pw", bufs=1 if CFG["expmerge"] else 2,
                         space="PSUM") as pw,
        ):
            # ---- constants ----
            # persistent logits slab for expmerge: two 1152-col halves,
            # alternated globally; each half holds a PAIR of subtiles so
            # exp runs once per pair (halves ACT per-op overhead)
            if CFG["expmerge"]:
                wpbig = pw.tile([128, 2304], F32, tag="wp")
            else:
                wpbig = None
            gpar = [0]  # global pair parity
            idxt = cpool.tile([128, nt * 8], I16, tag="idxt")
            nc.sync.dma_start(idxt[:], idx_d[:])
            cp = cpool.tile([128, 1473], BF16, tag="cpack")
            nc.sync.dma_start(cp[:], cpack_d[:])
            w3km0 = cp[:, 0:576]
            w3km1 = cp[:, 576:1152]
            w2t = cp[0:65, 1152:1408]
            w1t = cp[0:4, 1408:1473]

            xwin = x_windows_ap()

            GDEPTH = 2
            PDEPTH = 2
            gtiles = {}
            ptiles = {}

            def issue_pose(t):
                J = Js[t]
                _lbl(f'pose[{t}]')
                p3 = ppool.tile([4, JMAXT * 128], BF16, tag="p3")
                nc.sync.dma_start(
                    p3[:, 0:J * 128],
                    pose_d[:, base[t] * 128:(base[t] + J) * 128])
                ptiles[t] = p3

            def issue_gather(t):
                _lbl(f'gather[{t}]')
                g = gpool.tile([128, 1, 1152], BF16, tag="g")
                nc.gpsimd.dma_gather(
                    out_ap=g[:],
                    in_ap=xwin,
                    idxs_ap=idxt[:, t * 8:(t + 1) * 8],
                    num_idxs=128,
                    num_idxs_reg=128,
                    elem_size=1152,
                    elem_step=384,
                )
                gtiles[t] = g

            for t in range(min(PDEPTH, nt)):
                issue_pose(t)
            for t in range(min(GDEPTH, nt)):
                issue_gather(t)

            mlp_done = {}  # chunk idx -> (h1s, h2s)

            def mlp_thunks(i):
                """Per-q-chunk MLP thunks for chunk i (h1mm, h1relu, h2mms,
                h2relu); interleaved into the previous chunk's stage-1
                stream so evictions are ready early."""
                t, jb, jc, _ = chunks[i]
                p3 = ptiles[t]
                npx = jc * 128
                h1s = mpool.tile([65, JCAP * 128], BF16, tag="h1s")
                h2s = mpool.tile([128, 2, JCAP * 128], BF16, tag="h2s")
                mlp_done[i] = (h1s, h2s)

                def mk(q0):
                    def go():
                        qn = min(512, npx - q0)
                        qs = slice(jb * 128 + q0, jb * 128 + q0 + qn)
                        qd = slice(q0, q0 + qn)
                        _lbl(f'mlp[{i}]q{q0//512}')
                        h1p = ph1.tile([65, 512], F32, tag="h1p")
                        nc.tensor.matmul(h1p[:, 0:qn], w1t, p3[:, qs],
                                         start=True, stop=True)
                        nc.scalar.activation(h1s[:, qd], h1p[:, 0:qn],
                                             AF.Relu)
                        h2p = ph2.tile([128, 2, 512], F32, tag="h2p")
                        for cc in range(2):
                            nc.tensor.matmul(h2p[:, cc, 0:qn],
                                             w2t[:, cc * 128:(cc + 1) * 128],
                                             h1s[:, qd], start=True,
                                             stop=True)
                        m = CFG["h2relu"]
                        on_act = m == "act" or (m == "alt" and
                                                (q0 // 512) % 2 == 1)
                        if on_act:
                            nc.scalar.activation(h2s[:, :, qd],
                                                 h2p[:, :, 0:qn], AF.Relu)
                        else:
                            nc.gpsimd.tensor_scalar_max(h2s[:, :, qd],
                                                        h2p[:, :, 0:qn], 0.0)
                    return go

                return [mk(q0) for q0 in range(0, npx, 512)]

            stage2 = []

            def stage1_thunks(i):
                """Thunks for chunk i: per-subtile logits+exp, mul groups
                after every 4 exps, then d1/d2.  Appends to stage2."""
                t, jb, jc, slot0 = chunks[i]
                h1s, h2s = mlp_done.pop(i)
                g = gtiles[t]
                e_f = epool.tile([128, JCAP, 576], BF16, tag="e_t")
                e = e_f[:, 0:jc, :]
                g3 = g.rearrange("p o (k b c) -> p o k b c", k=9, b=2)
                prods = vpool.tile([128, JCAP, 9, 128], BF16, tag="prods")
                e5f = e.rearrange("p j (k c) -> p k j c", k=9)
                prvf = prods[:, 0:jc, :, :].rearrange(
                    "p j k (b c) -> p k j b c", b=2)

                def mk_logit_plain(s):
                    def go():
                        _lbl(f'logit[{i}]s{s}')
                        ss = slice(s * 128, s * 128 + 128)
                        wp = pw.tile([128, 576], F32, tag="wp")
                        for r0, r1 in ((0, 512), (512, 576)):
                            nc.tensor.matmul(wp[:, r0:r1], h2s[:, 0, ss],
                                             w3km0[:, r0:r1], start=True,
                                             stop=False)
                            nc.tensor.matmul(wp[:, r0:r1], h2s[:, 1, ss],
                                             w3km1[:, r0:r1], start=False,
                                             stop=True)
                        nc.scalar.activation(e[:, s, :], wp[:], AF.Exp)
                    return go

                def mk_logit_merged(s, half):
                    def go():
                        _lbl(f'logit[{i}]s{s}')
                        ss = slice(s * 128, s * 128 + 128)
                        c0 = half * 1152 + (s % 2) * 576
                        rsp = (c0 // 512 + 1) * 512
                        for r0, r1 in ((c0, rsp), (rsp, c0 + 576)):
                            nc.tensor.matmul(wpbig[:, r0:r1], h2s[:, 0, ss],
                                             w3km0[:, r0 - c0:r1 - c0],
                                             start=True, stop=False)
                            nc.tensor.matmul(wpbig[:, r0:r1], h2s[:, 1, ss],
                                             w3km1[:, r0 - c0:r1 - c0],
                                             start=False, stop=True)
                        if s % 2 == 1 or s + 1 == jc:
                            s0 = s - (s % 2)
                            nn = (s - s0 + 1) * 576
                            p0 = half * 1152
                            nc.scalar.activation(e[:, s0:s + 1, :],
                                                 wpbig[:, p0:p0 + nn], AF.Exp)
                    return go

                def mk_logit(s):
                    if not CFG["expmerge"]:
                        return mk_logit_plain(s)
                    if s % 2 == 0:
                        gpar[0] ^= 1
                    return mk_logit_merged(s, gpar[0])

                def mk_muls(j0, j1):
                    def go():
                        _lbl(f'mul[{i}]j{j0}')
                        pj0 = max(j0, min(CFG["mgrp"], j1))
                        pj1 = max(j0, min(CFG["mgrp"] + CFG["poolj"], j1))
                        for b in range(2):
                            gbf = g3[:, :, :, b, :].rearrange(
                                "p o k c -> p k (o c)").unsqueeze(2)
                            for a0, a1, eng in ((j0, pj0, nc.vector),
                                                (pj0, pj1, nc.gpsimd),
                                                (pj1, j1, nc.vector)):
                                if a0 >= a1:
                                    continue
                                gb = gbf.broadcast_to((128, 9, a1 - a0, 64))
                                eng.tensor_mul(prvf[:, :, a0:a1, b, :],
                                               gb, e5f[:, :, a0:a1, :])
                    return go

                def mk_d12(j0, j1):
                    def go():
                        _lbl(f'd12[{i}]')
                        eg = e[:, j0:j1, :]
                        nc.vector.tensor_add(eg[:, :, 0:256], eg[:, :, 0:256],
                                             eg[:, :, 256:512])
                        if CFG["d2"] == "dve":
                            nc.vector.tensor_add(eg[:, :, 0:128],
                                                 eg[:, :, 0:128],
                                                 eg[:, :, 128:256])
                    return go

                rec = (i, e, prods, jc, slot0)

                def mk_d1dma():
                    def go():
                        emit_d1dma(rec)
                    return go

                thunks = []
                j0 = 0
                for s in range(jc):
                    thunks.append(mk_logit(s))
                    if s + 1 == jc or (s + 1) % CFG['mgrp'] == 0:
                        thunks.append(mk_muls(j0, s + 1))
                        if CFG["d1"] == "dve":
                            thunks.append(mk_d12(j0, s + 1))
                        elif s + 1 == jc:
                            thunks.append(mk_d1dma())
                        thunks.append(None)  # slot for an MLP q-thunk
                        j0 = s + 1
                stage2.append(rec)
                return thunks

            def emit_d1dma(rec):
                # den level 1 on accumulating DMA: e[:,:,0:256] += e[:,:,256:512]
                i, e, prods, jc, slot0 = rec
                _lbl(f'd1dma[{i}]')
                nc.gpsimd.dma_start(e[:, :, 0:256], e[:, :, 256:512],
                                    accum_op=ADD)

            def emit_d2_dve(rec):
                i, e, prods, jc, slot0 = rec
                _lbl(f'd2[{i}]')
                nc.vector.tensor_add(e[:, :, 0:128], e[:, :, 0:128],
                                     e[:, :, 128:256])

            def emit_l2(rec):
                i, e, prods, jc, slot0 = rec
                _lbl(f'L2[{i}]')
                def ksl(a, b):
                    return prods[:, 0:jc, a:b, :].rearrange(
                        "p j k c -> p j (k c)")
                nc.gpsimd.dma_start(ksl(0, 2), ksl(2, 4), accum_op=ADD)
                if CFG["l3"] == "dma":
                    nc.gpsimd.dma_start(ksl(0, 1), ksl(1, 2), accum_op=ADD)

            def emit_l1(rec):
                i, e, prods, jc, slot0 = rec
                _lbl(f'L1[{i}]')
                def ksl(a, b):
                    return prods[:, 0:jc, a:b, :].rearrange(
                        "p j k c -> p j (k c)")
                nc.gpsimd.dma_start(ksl(0, 4), ksl(4, 8), accum_op=ADD)

            dens = {}

            def emit_d3fin(rec):
                # writes den to its own tile so e's buffer frees here (early
                # in the iteration) instead of at DVE's div at iteration end
                i, e, prods, jc, slot0 = rec
                _lbl(f'd3fin[{i}]')
                den_f = dpool.tile([128, JCAP, 64], BF16, tag="den")
                den = den_f[:, 0:jc, :]
                eng = nc.vector if CFG["d3fin"] == "dve" else nc.gpsimd
                eng.tensor_add(den, e[:, :, 0:64], e[:, :, 64:128])
                eng.tensor_add(den, den, e[:, :, 512:576])
                dens[i] = den

            def emit_acc_div(rec):
                i, e, prods, jc, slot0 = rec
                _lbl(f'accdiv[{i}]')
                k0 = prods[:, 0:jc, 0, :]
                if CFG["l3"] == "dve":
                    nc.vector.tensor_add(k0, k0, prods[:, 0:jc, 1, :])
                nc.vector.tensor_add(k0, k0, prods[:, 0:jc, 8, :])
                out_f = opool.tile([128, JCAP, 128], BF16, tag="out_t")
                out_t = out_f[:, 0:jc, :]
                ov = out_t.rearrange("p j (b c) -> p j b c", b=2)
                av = k0.rearrange("p j (b c) -> p j b c", b=2)
                den = dens.pop(i)
                rden_f = dpool.tile([128, JCAP, 64], BF16, tag="rden")
                rden = rden_f[:, 0:jc, :]
                with nc.allow_low_precision(reason="softmax denom in bf16"):
                    nc.vector.reciprocal(rden, den)
                dv = rden.unsqueeze(2).broadcast_to((128, jc, 2, 64))
                nc.vector.tensor_mul(ov, av, dv)
                nc.sync.dma_start(
                    out_d[:, slot0:slot0 + jc].rearrange("p s c -> p (s c)"),
                    out_t.rearrange("p s c -> p (s c)"))

            # pipeline per iteration i:
            #   Pool: d3fin(i-1), h2relus(i+1) (interleaved), L2(i-1), L1(i)
            #   PE:   logits(i) with MLP(i+1) q-blocks in the exp-wait gaps
            #   ACT:  exps(i) + h1relus(i+1) interleaved
            #   DVE:  mul+d12 groups(i), acc2+div(i-1)
            for th0 in mlp_thunks(0):
                th0()
            for i in range(nch):
                t = chunks[i][0]
                tn = chunks[i + 1][0] if i + 1 < nch else None
                if tn is not None and tn != t:
                    if tn + PDEPTH - 1 < nt:
                        issue_pose(tn + PDEPTH - 1)
                    if tn + GDEPTH - 1 < nt:
                        issue_gather(tn + GDEPTH - 1)
                prev = stage2.pop(0) if stage2 else None
                if prev is not None:
                    emit_d3fin(prev)
                    emit_l2(prev)
                th = stage1_thunks(i)
                mth = list(mlp_thunks(i + 1)) if i + 1 < nch else []
                mi = 0
                for thunk in th:
                    if thunk is None:
                        if mi < len(mth):
                            mth[mi]()
                            mi += 1
                    else:
                        thunk()
                while mi < len(mth):
                    mth[mi]()
                    mi += 1
                if prev is not None:
                    emit_acc_div(prev)
                if CFG["d1"] == "dma2":
                    # d2 for the CURRENT chunk at DVE's iteration end: the
                    # d1 accum-DMA (fired right after the muls) is complete
                    # by the time DVE drains its finish ops
                    emit_d2_dve(stage2[-1])
                emit_l1(stage2[-1])
                if CFG["d2"] == "pool":
                    # after L1 in Pool's queue: d2's dep (d1 on DVE) resolves
                    # later than L1's (muls), so this order avoids head-block
                    ii, ee, _pr, _jc, _sl = stage2[-1]
                    _lbl(f'd2p[{ii}]')
                    nc.gpsimd.tensor_add(ee[:, :, 0:128], ee[:, :, 0:128],
                                         ee[:, :, 128:256])
            # drain last chunk
            rec = stage2.pop(0)
            emit_d3fin(rec)
            emit_l2(rec)
            emit_acc_div(rec)

    nc.compile()
    return nc


def _schedule(interMapY, interMapX):
    Y = np.asarray(interMapY).astype(np.int64).reshape(-1)
    X = np.asarray(interMapX).astype(np.int64).reshape(-1)
    m = (Y * SCW + X).astype(np.int32)
    order = np.argsort(m, kind='stable')
    ms = m[order]
    uniq, first, inv_s, counts = np.unique(
        ms, return_index=True, return_inverse=True, return_counts=True)
    ncell = len(uniq)
    C8 = ((ncell + 1023) // 1024) * 1024
    padn = C8 - ncell
    counts_p = np.concatenate([counts, np.zeros(padn, counts.dtype)])
    uniq_p = np.concatenate([uniq, np.zeros(padn, uniq.dtype)])
    rank_order = np.argsort(-counts_p, kind='stable')  # cellidx by rank
    R = C8
    ranks = np.arange(R)
    blk, pos = ranks // 8, ranks % 8
    core_of_rank = np.where(blk % 2 == 0, pos, 7 - pos)
    percore_pos = np.zeros(R, np.int64)
    for c in range(NCORES):
        sel = core_of_rank == c
        percore_pos[sel] = np.arange(sel.sum())
    nt = (C8 // NCORES) // 128
    # smallest-J tiles first: shorter pipeline fill at kernel start
    tile_of = nt - 1 - percore_pos // 128
    part_of = percore_pos % 128
    cnt_r = counts_p[rank_order]
    Js = np.zeros(nt, np.int64)
    for t in range(nt):
        Js[t] = max(1, cnt_r[tile_of == t].max())
    base = np.concatenate([[0], np.cumsum(Js)])[:-1]
    ST = int(Js.sum())

    rank_of_cell = np.empty(R, np.int64)
    rank_of_cell[rank_order] = ranks
    r_px = rank_of_cell[inv_s]  # per sorted-pixel
    core_px = core_of_rank[r_px]
    slot_px = base[tile_of[r_px]] + (np.arange(PXTOT) - first[inv_s])
    part_px = part_of[r_px]

    win = np.zeros((NCORES, nt, 128), np.int64)
    win[core_of_rank, tile_of, part_of] = uniq_p[rank_order]
    return dict(order=order, core_px=core_px, slot_px=slot_px,
                part_px=part_px, Js=tuple(int(j) for j in Js), base=base,
                ST=ST, win=win, nt=nt)


def _host_prep(x, poseMap, W1, b1, W2, b2, W3, b3, sch):
    bf = ml_dtypes.bfloat16
    xp = np.pad(np.asarray(x, np.float32), ((0, 0), (0, 0), (1, 1), (1, 1)))
    xs = np.ascontiguousarray(np.transpose(xp, (2, 3, 0, 1)))
    sw = np.lib.stride_tricks.sliding_window_view(xs, 3, axis=0)
    xsc = np.ascontiguousarray(np.transpose(sw, (0, 1, 4, 2, 3)))
    xsc = xsc.reshape(NSC * 384).astype(bf)

    ST, nt = sch["ST"], sch["nt"]
    pose_full = np.concatenate(
        [np.asarray(poseMap, np.float32)[0].reshape(3, PXTOT),
         np.ones((1, PXTOT), np.float32)], axis=0)

    w1t32 = np.zeros((4, 65), np.float32)
    w1t32[0:3, 0:64] = np.asarray(W1, np.float32).T
    w1t32[3, 0:64] = np.asarray(b1, np.float32)
    w1t32[3, 64] = 1.0
    w2t32 = np.concatenate([np.asarray(W2, np.float32).T,
                            np.asarray(b2, np.float32)[None, :]], axis=0)
    W3r = np.asarray(W3, np.float32).reshape(C, KS, KS, 256)
    w3km32 = np.ascontiguousarray(
        np.transpose(W3r, (3, 2, 1, 0))).reshape(256, 576)
    # fold b3 into... b3 is zeros in this problem; assert to be safe
    b3a = np.asarray(b3, np.float32)

    cpack = np.zeros((128, 1473), np.float32)
    cpack[:, 0:576] = w3km32[0:128]
    cpack[:, 576:1152] = w3km32[128:256]
    cpack[0:65, 1152:1408] = w2t32
    cpack[0:4, 1408:1473] = w1t32
    cpack = cpack.astype(bf)

    in_maps = []
    for c in range(NCORES):
        sel = sch["core_px"] == c
        pose = np.zeros((4, ST * 128), np.float32)
        cols = sch["slot_px"][sel] * 128 + sch["part_px"][sel]
        pose[:, cols] = pose_full[:, sch["order"][sel]]
        ids = sch["win"][c]  # [nt, 128]
        idxw = np.zeros((128, nt * 8), np.int16)
        for t in range(nt):
            a = ids[t].astype(np.int16).reshape(8, 16)  # [cc, 16]
            idxw[:, t * 8:(t + 1) * 8] = np.tile(a.T, (8, 1))
        in_maps.append({
            "xw": xsc,
            "idx": np.ascontiguousarray(idxw),
            "pose": np.ascontiguousarray(pose).astype(bf),
            "cpack": cpack,
        })
    return in_maps, b3a


def kernel(**inputs):
    global LAST_RESULT, LAST_PROG
    sch = _schedule(inputs["interMapY"], inputs["interMapX"])
    key = sch["Js"]
    if key not in _PROG_CACHE:
        _PROG_CACHE[key] = build_program(sch["Js"])
    nc = _PROG_CACHE[key]
    LAST_PROG = nc
    in_maps, b3a = _host_prep(
        inputs["x"], inputs["poseMap"], inputs["W1"], inputs["b1"],
        inputs["W2"], inputs["b2"], inputs["W3"], inputs["b3"], sch)
    os.environ.setdefault("BASS_NEVER_TRACE", "1")
    res = None
    last_err = None
    for attempt in range(3):
        try:
            res = run_bass_kernel_spmd(nc, in_maps, list(range(NCORES)))
            break
        except Exception as err:
            last_err = err
            os.environ["NEURON_RT_RESET_CORES"] = "1"
    if res is None:
        raise last_err
    LAST_RESULT = res
    ST = sch["ST"]
    out_full = np.zeros((PXTOT, 128), np.float32)
    for c in range(NCORES):
        arr = np.asarray(res.results[c]["out"]).reshape(128, ST, 128)
        sel = sch["core_px"] == c
        out_full[sch["order"][sel]] = arr[
            sch["part_px"][sel], sch["slot_px"][sel], :].astype(np.float32)
    out = out_full.reshape(HO, WO, BS, C).transpose(2, 3, 0, 1)
    # b3 correction: b3 is zero in this problem's setup; softmax with b3
    # would change weights -- recompute would be needed.  Guard loudly.
    if np.any(b3a != 0.0):
        raise NotImplementedError("nonzero b3 not folded in this kernel")
    return np.ascontiguousarray(out)


if __name__ == "__main__":
    data = np.load(sys.argv[1] if len(sys.argv) > 1 else "work/inputs.npz")
    out = kernel(**{k: data[k] for k in data.files})
    print("out", out.shape, out.dtype, float(np.abs(out).max()))


# revision 55
# speedup vs baseline: 1.0344x; 1.0344x over previous
"""Trainium2 Bass kernel for nn_BilinearUpsampler (cell-sorted v3).

out[b,c,i,j] = sum_k softmax_k(MLP(poseMap)[c,k,i,j]) * xpad[b,c,Y[i,j]+dy_k,X[i,j]+dx_k]

Key idea vs the per-pixel-gather baseline: output pixels are grouped by
their source cell (Y,X).  A tile = 128 cells (on partitions) x J slots
(pixels of that cell, on the free axis).  The 3x3 input window (1152
bf16 = one "cell window") is gathered ONCE per cell (128 descriptors
per tile instead of one per pixel -> 16x less DMA) and broadcast over
the cell's pixels with a stride-0 AP in the tap multiply.  Cells are
occupancy-sorted into tiles so J per tile ~= every cell's count (pad
waste ~2-4%).  The schedule is data-dependent; the program is compiled
per schedule signature and cached.

Work placement (tuned against the TimelineSim cost model; PSUM reads
are only legal on PE/ACT/DVE, so all PSUM evictions sit on ACT):
  PE   : MLP (3->64->256) + logits (256->576)
  ACT  : h1/h2 relu evictions, exp eviction (PSUM->SBUF)
  DVE  : tap muls (bf16 2x, g broadcast over j via stride-0 AP), den
         d1/d2 (in-place on e), tree finish (k0+=k1, k0+=k8 in prods),
         reciprocal, final normalize multiply
  Pool : den d3/final into a separate den tile (frees e early -- this
         breaks the exp->...->div buffer cycle that serialized ACT
         behind DVE), SWDGE descriptor-gen for gathers and accum DMAs
  DMA  : per-cell window gather, accumulating SBUF->SBUF tap-tree
         levels L1 (k[0:4]+=k[4:8]) and L2 (k[0:2]+=k[2:4]), pose in,
         out store

Pipeline per iteration i (in-order engine queues head-block on the
first not-ready instruction, so per-engine emission order is chosen to
match dependency readiness): d2+den(i-1) -> logits/exp(i) subtiles with
tap-mul groups spliced in and MLP(i+1) q-blocks in the exp-wait gaps ->
L2(i-1) -> tree-finish+recip+normalize+store(i-1) -> L1(i).
"""

import sys
import os

sys.path.insert(0, "/opt/trn_rl_repo")

import numpy as np
import ml_dtypes

import concourse.bass as bass
import concourse.bacc as bacc
import concourse.mybir as mybir
import concourse.tile as tile
import concourse.alu_op_type as alu
from concourse.bass_utils import run_bass_kernel_spmd

BF16 = mybir.dt.bfloat16
F32 = mybir.dt.float32
I16 = mybir.dt.int16
AF = mybir.ActivationFunctionType
ADD = alu.AluOpType.add

NCORES = 8
C = 64
KS = 3
BS = 2
HI = WI = 128
HO = WO = 512
SCW = HI + 2  # supercell grid width (padded x: 0..129)
NSC = HI * SCW  # supercells
NWIN = NSC - 2  # gatherable 3-supercell windows
PXTOT = HO * WO

JCAP = 16  # max j-slots per chunk

# work-placement knobs (swept against the cost model)
CFG = {
    "d1": "dve",      # den level 1: 'dve' | 'dma'
    "d3fin": "pool",   # den level 3 + final: 'dve' | 'pool'
    "h2relu": "act",  # h2 relu eviction: 'act' (pool cannot read PSUM)
    "poolj": 0,       # j-slots per chunk whose tap-muls run on Pool
    "mgrp": 8,        # j-slots per tap-mul group
}

LAST_RESULT = None
LAST_PROG = None

_PROG_CACHE = {}

LABEL_HOOK = None


def _lbl(s):
    if LABEL_HOOK is not None:
        LABEL_HOOK(s)


def _chunks(J):
    nch = (J + JCAP - 1) // JCAP
    lo = J // nch
    hi = lo + 1
    nhi = J - lo * nch
    return [hi] * nhi + [lo] * (nch - nhi)


def build_program(Js):
    """Js: per-tile slot counts (shared across all 8 cores)."""
    Js = list(Js)
    nt = len(Js)
    ST = sum(Js)
    base = np.concatenate([[0], np.cumsum(Js)])[:-1]
    JMAXT = max(Js)

    nc = bacc.Bacc("TRN2", target_bir_lowering=False, debug=False,
                   num_devices=NCORES)

    xw_d = nc.dram_tensor("xw", [NSC * 384], BF16, kind="ExternalInput")
    idx_d = nc.dram_tensor("idx", [128, nt * 8], I16, kind="ExternalInput")
    pose_d = nc.dram_tensor("pose", [4, ST * 128], BF16, kind="ExternalInput")
    # cpack: [0:576] w3km0, [576:1152] w3km1, [1152:1408] w2t (rows 0-64),
    # [1408:1473] w1t (rows 0-3)
    cpack_d = nc.dram_tensor("cpack", [128, 1473], BF16, kind="ExternalInput")
    out_d = nc.dram_tensor("out", [128, ST, 128], BF16, kind="ExternalOutput")

    import bass_rust

    def x_windows_ap():
        ap = xw_d[:].copy()
        ap.ap = bass_rust.VecI64Pair([(384, NWIN), (1, 1152)])
        return ap

    # flat chunk list: (tile, jb, jc, slot0)
    chunks = []
    for t in range(nt):
        jb = 0
        for jc in _chunks(Js[t]):
            chunks.append((t, jb, jc, int(base[t]) + jb))
            jb += jc
    nch = len(chunks)

    with tile.TileContext(nc) as tc:
        with (
            tc.tile_pool(name="consts", bufs=1) as cpool,
            tc.tile_pool(name="pose", bufs=3) as ppool,
            tc.tile_pool(name="gath", bufs=3) as gpool,
            tc.tile_pool(name="mlp", bufs=2) as mpool,
            tc.tile_pool(name="ework", bufs=2) as epool,
            tc.tile_pool(name="prod", bufs=2) as vpool,
            tc.tile_pool(name="dpool", bufs=2) as dpool,
            tc.tile_pool(name="outp", bufs=2) as opool,
            tc.tile_pool(name="ph1", bufs=1 if CFG["expmerge"] else 2,
                         space="PSUM") as ph1,
            tc.tile_pool(name="ph2", bufs=1, space="PSUM") as ph2,
            tc.tile_pool(name="pw", bufs=1 if CFG["expmerge"] else 2,
                         space="PSUM") as pw,
        ):
            # ---- constants ----
            # persistent logits slab for expmerge: two 1152-col halves,
            # alternated globally; each half holds a PAIR of subtiles so
            # exp runs once per pair (halves ACT per-op overhead)
            if CFG["expmerge"]:
                wpbig = pw.tile([128, 2304], F32, tag="wp")
            else:
                wpbig = None
            gpar = [0]  # global pair parity
            idxt = cpool.tile([128, nt * 8], I16, tag="idxt")
            nc.sync.dma_start(idxt[:], idx_d[:])
            cp = cpool.tile([128, 1473], BF16, tag="cpack")
            nc.sync.dma_start(cp[:], cpack_d[:])
            w3km0 = cp[:, 0:576]
            w3km1 = cp[:, 576:1152]
            w2t = cp[0:65, 1152:1408]
            w1t = cp[0:4, 1408:1473]

            xwin = x_windows_ap()

            GDEPTH = 2
            PDEPTH = 2
            gtiles = {}
            ptiles = {}

            def issue_pose(t):
                J = Js[t]
                _lbl(f'pose[{t}]')
                p3 = ppool.tile([4, JMAXT * 128], BF16, tag="p3")
                nc.sync.dma_start(
                    p3[:, 0:J * 128],
                    pose_d[:, base[t] * 128:(base[t] + J) * 128])
                ptiles[t] = p3

            def issue_gather(t):
                _lbl(f'gather[{t}]')
                g = gpool.tile([128, 1, 1152], BF16, tag="g")
                nc.gpsimd.dma_gather(
                    out_ap=g[:],
                    in_ap=xwin,
                    idxs_ap=idxt[:, t * 8:(t + 1) * 8],
                    num_idxs=128,
                    num_idxs_reg=128,
                    elem_size=1152,
                    elem_step=384,
                )
                gtiles[t] = g

            for t in range(min(PDEPTH, nt)):
                issue_pose(t)
            for t in range(min(GDEPTH, nt)):
                issue_gather(t)

            mlp_done = {}  # chunk idx -> (h1s, h2s)

            def mlp_thunks(i):
                """Per-q-chunk MLP thunks for chunk i (h1mm, h1relu, h2mms,
                h2relu); interleaved into the previous chunk's stage-1
                stream so evictions are ready early."""
                t, jb, jc, _ = chunks[i]
                p3 = ptiles[t]
                npx = jc * 128
                h1s = mpool.tile([65, JCAP * 128], BF16, tag="h1s")
                h2s = mpool.tile([128, 2, JCAP * 128], BF16, tag="h2s")
                mlp_done[i] = (h1s, h2s)

                def mk(q0):
                    def go():
                        qn = min(512, npx - q0)
                        qs = slice(jb * 128 + q0, jb * 128 + q0 + qn)
                        qd = slice(q0, q0 + qn)
                        _lbl(f'mlp[{i}]q{q0//512}')
                        h1p = ph1.tile([65, 512], F32, tag="h1p")
                        nc.tensor.matmul(h1p[:, 0:qn], w1t, p3[:, qs],
                                         start=True, stop=True)
                        nc.scalar.activation(h1s[:, qd], h1p[:, 0:qn],
                                             AF.Relu)
                        h2p = ph2.tile([128, 2, 512], F32, tag="h2p")
                        for cc in range(2):
                            nc.tensor.matmul(h2p[:, cc, 0:qn],
                                             w2t[:, cc * 128:(cc + 1) * 128],
                                             h1s[:, qd], start=True,
                                             stop=True)
                        m = CFG["h2relu"]
                        on_act = m == "act" or (m == "alt" and
                                                (q0 // 512) % 2 == 1)
                        if on_act:
                            nc.scalar.activation(h2s[:, :, qd],
                                                 h2p[:, :, 0:qn], AF.Relu)
                        else:
                            nc.gpsimd.tensor_scalar_max(h2s[:, :, qd],
                                                        h2p[:, :, 0:qn], 0.0)
                    return go

                return [mk(q0) for q0 in range(0, npx, 512)]

            stage2 = []

            def stage1_thunks(i):
                """Thunks for chunk i: per-subtile logits+exp, mul groups
                after every 4 exps, then d1/d2.  Appends to stage2."""
                t, jb, jc, slot0 = chunks[i]
                h1s, h2s = mlp_done.pop(i)
                g = gtiles[t]
                e_f = epool.tile([128, JCAP, 576], BF16, tag="e_t")
                e = e_f[:, 0:jc, :]
                g3 = g.rearrange("p o (k b c) -> p o k b c", k=9, b=2)
                prods = vpool.tile([128, JCAP, 9, 128], BF16, tag="prods")
                e5f = e.rearrange("p j (k c) -> p k j c", k=9)
                prvf = prods[:, 0:jc, :, :].rearrange(
                    "p j k (b c) -> p k j b c", b=2)

                def mk_logit_plain(s):
                    def go():
                        _lbl(f'logit[{i}]s{s}')
                        ss = slice(s * 128, s * 128 + 128)
                        wp = pw.tile([128, 576], F32, tag="wp")
                        for r0, r1 in ((0, 512), (512, 576)):
                            nc.tensor.matmul(wp[:, r0:r1], h2s[:, 0, ss],
                                             w3km0[:, r0:r1], start=True,
                                             stop=False)
                            nc.tensor.matmul(wp[:, r0:r1], h2s[:, 1, ss],
                                             w3km1[:, r0:r1], start=False,
                                             stop=True)
                        nc.scalar.activation(e[:, s, :], wp[:], AF.Exp)
                    return go

                def mk_logit_merged(s, half):
                    def go():
                        _lbl(f'logit[{i}]s{s}')
                        ss = slice(s * 128, s * 128 + 128)
                        c0 = half * 1152 + (s % 2) * 576
                        rsp = (c0 // 512 + 1) * 512
                        for r0, r1 in ((c0, rsp), (rsp, c0 + 576)):
                            nc.tensor.matmul(wpbig[:, r0:r1], h2s[:, 0, ss],
                                             w3km0[:, r0 - c0:r1 - c0],
                                             start=True, stop=False)
                            nc.tensor.matmul(wpbig[:, r0:r1], h2s[:, 1, ss],
                                             w3km1[:, r0 - c0:r1 - c0],
                                             start=False, stop=True)
                        if s % 2 == 1 or s + 1 == jc:
                            s0 = s - (s % 2)
                            nn = (s - s0 + 1) * 576
                            p0 = half * 1152
                            nc.scalar.activation(e[:, s0:s + 1, :],
                                                 wpbig[:, p0:p0 + nn], AF.Exp)
                    return go

                def mk_logit(s):
                    if not CFG["expmerge"]:
                        return mk_logit_plain(s)
                    if s % 2 == 0:
                        gpar[0] ^= 1
                    return mk_logit_merged(s, gpar[0])

                def mk_muls(j0, j1):
                    def go():
                        _lbl(f'mul[{i}]j{j0}')
                        pj0 = max(j0, min(CFG["mgrp"], j1))
                        pj1 = max(j0, min(CFG["mgrp"] + CFG["poolj"], j1))
                        for b in range(2):
                            gbf = g3[:, :, :, b, :].rearrange(
                                "p o k c -> p k (o c)").unsqueeze(2)
                            for a0, a1, eng in ((j0, pj0, nc.vector),
                                                (pj0, pj1, nc.gpsimd),
                                                (pj1, j1, nc.vector)):
                                if a0 >= a1:
                                    continue
                                gb = gbf.broadcast_to((128, 9, a1 - a0, 64))
                                eng.tensor_mul(prvf[:, :, a0:a1, b, :],
                                               gb, e5f[:, :, a0:a1, :])
                    return go

                def mk_d12(j0, j1):
                    def go():
                        _lbl(f'd12[{i}]')
                        eg = e[:, j0:j1, :]
                        nc.vector.tensor_add(eg[:, :, 0:256], eg[:, :, 0:256],
                                             eg[:, :, 256:512])
                        if CFG["d2"] == "dve":
                            nc.vector.tensor_add(eg[:, :, 0:128],
                                                 eg[:, :, 0:128],
                                                 eg[:, :, 128:256])
                    return go

                rec = (i, e, prods, jc, slot0)

                def mk_d1dma():
                    def go():
                        emit_d1dma(rec)
                    return go

                thunks = []
                j0 = 0
                for s in range(jc):
                    thunks.append(mk_logit(s))
                    if s + 1 == jc or (s + 1) % CFG['mgrp'] == 0:
                        thunks.append(mk_muls(j0, s + 1))
                        if CFG["d1"] == "dve":
                            thunks.append(mk_d12(j0, s + 1))
                        elif s + 1 == jc:
                            thunks.append(mk_d1dma())
                        thunks.append(None)  # slot for an MLP q-thunk
                        j0 = s + 1
                stage2.append(rec)
                return thunks

            def emit_d1dma(rec):
                # den level 1 on accumulating DMA: e[:,:,0:256] += e[:,:,256:512]
                i, e, prods, jc, slot0 = rec
                _lbl(f'd1dma[{i}]')
                nc.gpsimd.dma_start(e[:, :, 0:256], e[:, :, 256:512],
                                    accum_op=ADD)

            def emit_d2_dve(rec):
                i, e, prods, jc, slot0 = rec
                _lbl(f'd2[{i}]')
                nc.vector.tensor_add(e[:, :, 0:128], e[:, :, 0:128],
                                     e[:, :, 128:256])

            def emit_l2(rec):
                i, e, prods, jc, slot0 = rec
                _lbl(f'L2[{i}]')
                def ksl(a, b):
                    return prods[:, 0:jc, a:b, :].rearrange(
                        "p j k c -> p j (k c)")
                nc.gpsimd.dma_start(ksl(0, 2), ksl(2, 4), accum_op=ADD)
                if CFG["l3"] == "dma":
                    nc.gpsimd.dma_start(ksl(0, 1), ksl(1, 2), accum_op=ADD)

            def emit_l1(rec):
                i, e, prods, jc, slot0 = rec
                _lbl(f'L1[{i}]')
                def ksl(a, b):
                    return prods[:, 0:jc, a:b, :].rearrange(
                        "p j k c -> p j (k c)")
                nc.gpsimd.dma_start(ksl(0, 4), ksl(4, 8), accum_op=ADD)

            dens = {}

            def emit_d3fin(rec):
                # writes den to its own tile so e's buffer frees here (early
                # in the iteration) instead of at DVE's div at iteration end
                i, e, prods, jc, slot0 = rec
                _lbl(f'd3fin[{i}]')
                den_f = dpool.tile([128, JCAP, 64], BF16, tag="den")
                den = den_f[:, 0:jc, :]
                eng = nc.vector if CFG["d3fin"] == "dve" else nc.gpsimd
                eng.tensor_add(den, e[:, :, 0:64], e[:, :, 64:128])
                eng.tensor_add(den, den, e[:, :, 512:576])
                dens[i] = den

            def emit_acc_div(rec):
                i, e, prods, jc, slot0 = rec
                _lbl(f'accdiv[{i}]')
                k0 = prods[:, 0:jc, 0, :]
                if CFG["l3"] == "dve":
                    nc.vector.tensor_add(k0, k0, prods[:, 0:jc, 1, :])
                nc.vector.tensor_add(k0, k0, prods[:, 0:jc, 8, :])
                out_f = opool.tile([128, JCAP, 128], BF16, tag="out_t")
                out_t = out_f[:, 0:jc, :]
                ov = out_t.rearrange("p j (b c) -> p j b c", b=2)
                av = k0.rearrange("p j (b c) -> p j b c", b=2)
                den = dens.pop(i)
                rden_f = dpool.tile([128, JCAP, 64], BF16, tag="rden")
                rden = rden_f[:, 0:jc, :]
                with nc.allow_low_precision(reason="softmax denom in bf16"):
                    nc.vector.reciprocal(rden, den)
                dv = rden.unsqueeze(2).broadcast_to((128, jc, 2, 64))
                nc.vector.tensor_mul(ov, av, dv)
                nc.sync.dma_start(
                    out_d[:, slot0:slot0 + jc].rearrange("p s c -> p (s c)"),
                    out_t.rearrange("p s c -> p (s c)"))

            # pipeline per iteration i:
            #   Pool: d3fin(i-1), h2relus(i+1) (interleaved), L2(i-1), L1(i)
            #   PE:   logits(i) with MLP(i+1) q-blocks in the exp-wait gaps
            #   ACT:  exps(i) + h1relus(i+1) interleaved
            #   DVE:  mul+d12 groups(i), acc2+div(i-1)
            for th0 in mlp_thunks(0):
                th0()
            for i in range(nch):
                t = chunks[i][0]
                tn = chunks[i + 1][0] if i + 1 < nch else None
                if tn is not None and tn != t:
                    if tn + PDEPTH - 1 < nt:
                        issue_pose(tn + PDEPTH - 1)
                    if tn + GDEPTH - 1 < nt:
                        issue_gather(tn + GDEPTH - 1)
                prev = stage2.pop(0) if stage2 else None
                if prev is not None:
                    emit_d3fin(prev)
                    emit_l2(prev)
                th = stage1_thunks(i)
                mth = list(mlp_thunks(i + 1)) if i + 1 < nch else []
                mi = 0
                for thunk in th:
                    if thunk is None:
                        if mi < len(mth):
                            mth[mi]()
                            mi += 1
                    else:
                        thunk()
                while mi < len(mth):
                    mth[mi]()
                    mi += 1
                if prev is not None:
                    emit_acc_div(prev)
                if CFG["d1"] == "dma2":
                    # d2 for the CURRENT chunk at DVE's iteration end: the
                    # d1 accum-DMA (fired right after the muls) is complete
                    # by the time DVE drains its finish ops
                    emit_d2_dve(stage2[-1])
                emit_l1(stage2[-1])
                if CFG["d2"] == "pool":
                    # after L1 in Pool's queue: d2's dep (d1 on DVE) resolves
                    # later than L1's (muls), so this order avoids head-block
                    ii, ee, _pr, _jc, _sl = stage2[-1]
                    _lbl(f'd2p[{ii}]')
                    nc.gpsimd.tensor_add(ee[:, :, 0:128], ee[:, :, 0:128],
                                         ee[:, :, 128:256])
            # drain last chunk
            rec = stage2.pop(0)
            emit_d3fin(rec)
            emit_l2(rec)
            emit_acc_div(rec)

    nc.compile()
    return nc


def _schedule(interMapY, interMapX):
    Y = np.asarray(interMapY).astype(np.int64).reshape(-1)
    X = np.asarray(interMapX).astype(np.int64).reshape(-1)
    m = (Y * SCW + X).astype(np.int32)
    order = np.argsort(m, kind='stable')
    ms = m[order]
    uniq, first, inv_s, counts = np.unique(
        ms, return_index=True, return_inverse=True, return_counts=True)
    ncell = len(uniq)
    # split cells with >JCAP pixels into <=JCAP-pixel pseudo-cells (each
    # segment re-gathers the same window; gathers are cheap).  All counts
    # then fit a single chunk, tiles get near-zero occupancy spread, and
    # ragged multi-chunk tiles disappear.
    nseg = (counts + JCAP - 1) // JCAP
    segbase = np.concatenate([[0], np.cumsum(nseg)])[:-1]
    P = int(nseg.sum())
    pc_cell = np.repeat(np.arange(ncell), nseg)
    seg_idx = np.arange(P) - np.repeat(segbase, nseg)
    counts = np.minimum(counts[pc_cell] - JCAP * seg_idx, JCAP)
    uniq = uniq[pc_cell]
    off_px = np.arange(PXTOT) - first[inv_s]
    pp_px = segbase[inv_s] + off_px // JCAP  # pseudo-cell per sorted pixel
    j_px = off_px % JCAP
    ncell = P
    C8 = ((ncell + 1023) // 1024) * 1024
    padn = C8 - ncell
    counts_p = np.concatenate([counts, np.zeros(padn, counts.dtype)])
    uniq_p = np.concatenate([uniq, np.zeros(padn, uniq.dtype)])
    rank_order = np.argsort(-counts_p, kind='stable')  # cellidx by rank
    R = C8
    ranks = np.arange(R)
    blk, pos = ranks // 8, ranks % 8
    core_of_rank = np.where(blk % 2 == 0, pos, 7 - pos)
    percore_pos = np.zeros(R, np.int64)
    for c in range(NCORES):
        sel = core_of_rank == c
        percore_pos[sel] = np.arange(sel.sum())
    nt = (C8 // NCORES) // 128
    # smallest-J tiles first: shorter pipeline fill at kernel start
    tile_of = nt - 1 - percore_pos // 128
    part_of = percore_pos % 128
    cnt_r = counts_p[rank_order]
    Js = np.zeros(nt, np.int64)
    for t in range(nt):
        Js[t] = max(1, cnt_r[tile_of == t].max())
    base = np.concatenate([[0], np.cumsum(Js)])[:-1]
    ST = int(Js.sum())

    rank_of_cell = np.empty(R, np.int64)
    rank_of_cell[rank_order] = ranks
    r_px = rank_of_cell[pp_px]  # per sorted-pixel (pseudo-cell rank)
    core_px = core_of_rank[r_px]
    slot_px = base[tile_of[r_px]] + j_px
    part_px = part_of[r_px]

    win = np.zeros((NCORES, nt, 128), np.int64)
    win[core_of_rank, tile_of, part_of] = uniq_p[rank_order]
    return dict(order=order, core_px=core_px, slot_px=slot_px,
                part_px=part_px, Js=tuple(int(j) for j in Js), base=base,
                ST=ST, win=win, nt=nt)


def _host_prep(x, poseMap, W1, b1, W2, b2, W3, b3, sch):
    bf = ml_dtypes.bfloat16
    xp = np.pad(np.asarray(x, np.float32), ((0, 0), (0, 0), (1, 1), (1, 1)))
    xs = np.ascontiguousarray(np.transpose(xp, (2, 3, 0, 1)))
    sw = np.lib.stride_tricks.sliding_window_view(xs, 3, axis=0)
    xsc = np.ascontiguousarray(np.transpose(sw, (0, 1, 4, 2, 3)))
    xsc = xsc.reshape(NSC * 384).astype(bf)

    ST, nt = sch["ST"], sch["nt"]
    pose_full = np.concatenate(
        [np.asarray(poseMap, np.float32)[0].reshape(3, PXTOT),
         np.ones((1, PXTOT), np.float32)], axis=0)

    w1t32 = np.zeros((4, 65), np.float32)
    w1t32[0:3, 0:64] = np.asarray(W1, np.float32).T
    w1t32[3, 0:64] = np.asarray(b1, np.float32)
    w1t32[3, 64] = 1.0
    w2t32 = np.concatenate([np.asarray(W2, np.float32).T,
                            np.asarray(b2, np.float32)[None, :]], axis=0)
    W3r = np.asarray(W3, np.float32).reshape(C, KS, KS, 256)
    w3km32 = np.ascontiguousarray(
        np.transpose(W3r, (3, 2, 1, 0))).reshape(256, 576)
    # fold b3 into... b3 is zeros in this problem; assert to be safe
    b3a = np.asarray(b3, np.float32)

    cpack = np.zeros((128, 1473), np.float32)
    cpack[:, 0:576] = w3km32[0:128]
    cpack[:, 576:1152] = w3km32[128:256]
    cpack[0:65, 1152:1408] = w2t32
    cpack[0:4, 1408:1473] = w1t32
    cpack = cpack.astype(bf)

    in_maps = []
    for c in range(NCORES):
        sel = sch["core_px"] == c
        pose = np.zeros((4, ST * 128), np.float32)
        cols = sch["slot_px"][sel] * 128 + sch["part_px"][sel]
        pose[:, cols] = pose_full[:, sch["order"][sel]]
        ids = sch["win"][c]  # [nt, 128]
        idxw = np.zeros((128, nt * 8), np.int16)
        for t in range(nt):
            a = ids[t].astype(np.int16).reshape(8, 16)  # [cc, 16]
            idxw[:, t * 8:(t + 1) * 8] = np.tile(a.T, (8, 1))
        in_maps.append({
            "xw": xsc,
            "idx": np.ascontiguousarray(idxw),
            "pose": np.ascontiguousarray(pose).astype(bf),
            "cpack": cpack,
        })
    return in_maps, b3a


def kernel(**inputs):
    global LAST_RESULT, LAST_PROG
    sch = _schedule(inputs["interMapY"], inputs["interMapX"])
    key = sch["Js"]
    if key not in _PROG_CACHE:
        _PROG_CACHE[key] = build_program(sch["Js"])
    nc = _PROG_CACHE[key]
    LAST_PROG = nc
    in_maps, b3a = _host_prep(
        inputs["x"], inputs["poseMap"], inputs["W1"], inputs["b1"],
        inputs["W2"], inputs["b2"], inputs["W3"], inputs["b3"], sch)
    os.environ.setdefault("BASS_NEVER_TRACE", "1")
    res = None
    last_err = None
    for attempt in range(3):
        try:
            res = run_bass_kernel_spmd(nc, in_maps, list(range(NCORES)))
            break
        except Exception as err:
            last_err = err
            os.environ["NEURON_RT_RESET_CORES"] = "1"
    if res is None:
        raise last_err
    LAST_RESULT = res
    ST = sch["ST"]
    out_full = np.zeros((PXTOT, 128), np.float32)
    for c in range(NCORES):
        arr = np.asarray(res.results[c]["out"]).reshape(128, ST, 128)
        sel = sch["core_px"] == c
        out_full[sch["order"][sel]] = arr[
            sch["part_px"][sel], sch["slot_px"][sel], :].astype(np.float32)
    out = out_full.reshape(HO, WO, BS, C).transpose(2, 3, 0, 1)
    # b3 correction: b3 is zero in this problem's setup; softmax with b3
    # would change weights -- recompute would be needed.  Guard loudly.
    if np.any(b3a != 0.0):
        raise NotImplementedError("nonzero b3 not folded in this kernel")
    return np.ascontiguousarray(out)


if __name__ == "__main__":
    data = np.load(sys.argv[1] if len(sys.argv) > 1 else "work/inputs.npz")
    out = kernel(**{k: data[k] for k in data.files})
    print("out", out.shape, out.dtype, float(np.abs(out).max()))


# revision 56
# speedup vs baseline: 1.0395x; 1.0050x over previous
"""Trainium2 Bass kernel for nn_BilinearUpsampler (cell-sorted v3).

out[b,c,i,j] = sum_k softmax_k(MLP(poseMap)[c,k,i,j]) * xpad[b,c,Y[i,j]+dy_k,X[i,j]+dx_k]

Key idea vs the per-pixel-gather baseline: output pixels are grouped by
their source cell (Y,X).  A tile = 128 cells (on partitions) x J slots
(pixels of that cell, on the free axis).  The 3x3 input window (1152
bf16 = one "cell window") is gathered ONCE per cell (128 descriptors
per tile instead of one per pixel -> 16x less DMA) and broadcast over
the cell's pixels with a stride-0 AP in the tap multiply.  Cells are
occupancy-sorted into tiles so J per tile ~= every cell's count (pad
waste ~2-4%).  The schedule is data-dependent; the program is compiled
per schedule signature and cached.

Work placement (tuned against the TimelineSim cost model; PSUM reads
are only legal on PE/ACT/DVE, so all PSUM evictions sit on ACT):
  PE   : MLP (3->64->256) + logits (256->576)
  ACT  : h1/h2 relu evictions, exp eviction (PSUM->SBUF)
  DVE  : tap muls (bf16 2x, g broadcast over j via stride-0 AP), den
         d1/d2 (in-place on e), tree finish (k0+=k1, k0+=k8 in prods),
         reciprocal, final normalize multiply
  Pool : den d3/final into a separate den tile (frees e early -- this
         breaks the exp->...->div buffer cycle that serialized ACT
         behind DVE), SWDGE descriptor-gen for gathers and accum DMAs
  DMA  : per-cell window gather, accumulating SBUF->SBUF tap-tree
         levels L1 (k[0:4]+=k[4:8]) and L2 (k[0:2]+=k[2:4]), pose in,
         out store

Pipeline per iteration i (in-order engine queues head-block on the
first not-ready instruction, so per-engine emission order is chosen to
match dependency readiness): d2+den(i-1) -> logits/exp(i) subtiles with
tap-mul groups spliced in and MLP(i+1) q-blocks in the exp-wait gaps ->
L2(i-1) -> tree-finish+recip+normalize+store(i-1) -> L1(i).
"""

import sys
import os

sys.path.insert(0, "/opt/trn_rl_repo")

import numpy as np
import ml_dtypes

import concourse.bass as bass
import concourse.bacc as bacc
import concourse.mybir as mybir
import concourse.tile as tile
import concourse.alu_op_type as alu
from concourse.bass_utils import run_bass_kernel_spmd

BF16 = mybir.dt.bfloat16
F32 = mybir.dt.float32
I16 = mybir.dt.int16
AF = mybir.ActivationFunctionType
ADD = alu.AluOpType.add

NCORES = 8
C = 64
KS = 3
BS = 2
HI = WI = 128
HO = WO = 512
SCW = HI + 2  # supercell grid width (padded x: 0..129)
NSC = HI * SCW  # supercells
NWIN = NSC - 2  # gatherable 3-supercell windows
PXTOT = HO * WO

JCAP = 16  # max j-slots per chunk

# work-placement knobs (swept against the cost model)
CFG = {
    "d1": "dve",      # den level 1: 'dve' | 'dma'
    "d3fin": "pool",   # den level 3 + final: 'dve' | 'pool'
    "h2relu": "act",  # h2 relu eviction: 'act' (pool cannot read PSUM)
    "poolj": 0,       # j-slots per chunk whose tap-muls run on Pool
    "mgrp": 8,        # j-slots per tap-mul group
}

LAST_RESULT = None
LAST_PROG = None

_PROG_CACHE = {}

LABEL_HOOK = None


def _lbl(s):
    if LABEL_HOOK is not None:
        LABEL_HOOK(s)


def _chunks(J):
    nch = (J + JCAP - 1) // JCAP
    lo = J // nch
    hi = lo + 1
    nhi = J - lo * nch
    return [hi] * nhi + [lo] * (nch - nhi)


def build_program(Js):
    """Js: per-tile slot counts (shared across all 8 cores)."""
    Js = list(Js)
    nt = len(Js)
    ST = sum(Js)
    base = np.concatenate([[0], np.cumsum(Js)])[:-1]
    JMAXT = max(Js)

    nc = bacc.Bacc("TRN2", target_bir_lowering=False, debug=False,
                   num_devices=NCORES)

    xw_d = nc.dram_tensor("xw", [NSC * 384], BF16, kind="ExternalInput")
    idx_d = nc.dram_tensor("idx", [128, nt * 8], I16, kind="ExternalInput")
    pose_d = nc.dram_tensor("pose", [4, ST * 128], BF16, kind="ExternalInput")
    # cpack: [0:576] w3km0, [576:1152] w3km1, [1152:1408] w2t (rows 0-64),
    # [1408:1473] w1t (rows 0-3)
    cpack_d = nc.dram_tensor("cpack", [128, 1473], BF16, kind="ExternalInput")
    out_d = nc.dram_tensor("out", [128, ST, 128], BF16, kind="ExternalOutput")

    import bass_rust

    def x_windows_ap():
        ap = xw_d[:].copy()
        ap.ap = bass_rust.VecI64Pair([(384, NWIN), (1, 1152)])
        return ap

    # flat chunk list: (tile, jb, jc, slot0)
    chunks = []
    for t in range(nt):
        jb = 0
        for jc in _chunks(Js[t]):
            chunks.append((t, jb, jc, int(base[t]) + jb))
            jb += jc
    nch = len(chunks)

    with tile.TileContext(nc) as tc:
        with (
            tc.tile_pool(name="consts", bufs=1) as cpool,
            tc.tile_pool(name="pose", bufs=3) as ppool,
            tc.tile_pool(name="gath", bufs=3) as gpool,
            tc.tile_pool(name="mlp", bufs=2) as mpool,
            tc.tile_pool(name="ework", bufs=2) as epool,
            tc.tile_pool(name="prod", bufs=2) as vpool,
            tc.tile_pool(name="dpool", bufs=2) as dpool,
            tc.tile_pool(name="outp", bufs=2) as opool,
            tc.tile_pool(name="ph1", bufs=1 if CFG["expmerge"] else 2,
                         space="PSUM") as ph1,
            tc.tile_pool(name="ph2", bufs=1, space="PSUM") as ph2,
            tc.tile_pool(name="pw", bufs=1 if CFG["expmerge"] else 2,
                         space="PSUM") as pw,
        ):
            # ---- constants ----
            # persistent logits slab for expmerge: two 1152-col halves,
            # alternated globally; each half holds a PAIR of subtiles so
            # exp runs once per pair (halves ACT per-op overhead)
            if CFG["expmerge"]:
                wpbig = pw.tile([128, 2304], F32, tag="wp")
            else:
                wpbig = None
            gpar = [0]  # global pair parity
            idxt = cpool.tile([128, nt * 8], I16, tag="idxt")
            nc.sync.dma_start(idxt[:], idx_d[:])
            cp = cpool.tile([128, 1473], BF16, tag="cpack")
            nc.sync.dma_start(cp[:], cpack_d[:])
            w3km0 = cp[:, 0:576]
            w3km1 = cp[:, 576:1152]
            w2t = cp[0:65, 1152:1408]
            w1t = cp[0:4, 1408:1473]

            xwin = x_windows_ap()

            GDEPTH = 2
            PDEPTH = 2
            gtiles = {}
            ptiles = {}

            def issue_pose(t):
                J = Js[t]
                _lbl(f'pose[{t}]')
                p3 = ppool.tile([4, JMAXT * 128], BF16, tag="p3")
                nc.sync.dma_start(
                    p3[:, 0:J * 128],
                    pose_d[:, base[t] * 128:(base[t] + J) * 128])
                ptiles[t] = p3

            def issue_gather(t):
                _lbl(f'gather[{t}]')
                g = gpool.tile([128, 1, 1152], BF16, tag="g")
                nc.gpsimd.dma_gather(
                    out_ap=g[:],
                    in_ap=xwin,
                    idxs_ap=idxt[:, t * 8:(t + 1) * 8],
                    num_idxs=128,
                    num_idxs_reg=128,
                    elem_size=1152,
                    elem_step=384,
                )
                gtiles[t] = g

            for t in range(min(PDEPTH, nt)):
                issue_pose(t)
            for t in range(min(GDEPTH, nt)):
                issue_gather(t)

            mlp_done = {}  # chunk idx -> (h1s, h2s)

            def mlp_thunks(i):
                """Per-q-chunk MLP thunks for chunk i (h1mm, h1relu, h2mms,
                h2relu); interleaved into the previous chunk's stage-1
                stream so evictions are ready early."""
                t, jb, jc, _ = chunks[i]
                p3 = ptiles[t]
                npx = jc * 128
                h1s = mpool.tile([65, JCAP * 128], BF16, tag="h1s")
                h2s = mpool.tile([128, 2, JCAP * 128], BF16, tag="h2s")
                mlp_done[i] = (h1s, h2s)

                def mk(q0):
                    def go():
                        qn = min(512, npx - q0)
                        qs = slice(jb * 128 + q0, jb * 128 + q0 + qn)
                        qd = slice(q0, q0 + qn)
                        _lbl(f'mlp[{i}]q{q0//512}')
                        h1p = ph1.tile([65, 512], F32, tag="h1p")
                        nc.tensor.matmul(h1p[:, 0:qn], w1t, p3[:, qs],
                                         start=True, stop=True)
                        nc.scalar.activation(h1s[:, qd], h1p[:, 0:qn],
                                             AF.Relu)
                        h2p = ph2.tile([128, 2, 512], F32, tag="h2p")
                        for cc in range(2):
                            nc.tensor.matmul(h2p[:, cc, 0:qn],
                                             w2t[:, cc * 128:(cc + 1) * 128],
                                             h1s[:, qd], start=True,
                                             stop=True)
                        m = CFG["h2relu"]
                        on_act = m == "act" or (m == "alt" and
                                                (q0 // 512) % 2 == 1)
                        if on_act:
                            nc.scalar.activation(h2s[:, :, qd],
                                                 h2p[:, :, 0:qn], AF.Relu)
                        else:
                            nc.gpsimd.tensor_scalar_max(h2s[:, :, qd],
                                                        h2p[:, :, 0:qn], 0.0)
                    return go

                return [mk(q0) for q0 in range(0, npx, 512)]

            stage2 = []

            def stage1_thunks(i):
                """Thunks for chunk i: per-subtile logits+exp, mul groups
                after every 4 exps, then d1/d2.  Appends to stage2."""
                t, jb, jc, slot0 = chunks[i]
                h1s, h2s = mlp_done.pop(i)
                g = gtiles[t]
                e_f = epool.tile([128, JCAP, 576], BF16, tag="e_t")
                e = e_f[:, 0:jc, :]
                g3 = g.rearrange("p o (k b c) -> p o k b c", k=9, b=2)
                prods = vpool.tile([128, JCAP, 9, 128], BF16, tag="prods")
                e5f = e.rearrange("p j (k c) -> p k j c", k=9)
                prvf = prods[:, 0:jc, :, :].rearrange(
                    "p j k (b c) -> p k j b c", b=2)

                def mk_logit_plain(s):
                    def go():
                        _lbl(f'logit[{i}]s{s}')
                        ss = slice(s * 128, s * 128 + 128)
                        wp = pw.tile([128, 576], F32, tag="wp")
                        for r0, r1 in ((0, 512), (512, 576)):
                            nc.tensor.matmul(wp[:, r0:r1], h2s[:, 0, ss],
                                             w3km0[:, r0:r1], start=True,
                                             stop=False)
                            nc.tensor.matmul(wp[:, r0:r1], h2s[:, 1, ss],
                                             w3km1[:, r0:r1], start=False,
                                             stop=True)
                        nc.scalar.activation(e[:, s, :], wp[:], AF.Exp)
                    return go

                def mk_logit_merged(s, half):
                    def go():
                        _lbl(f'logit[{i}]s{s}')
                        ss = slice(s * 128, s * 128 + 128)
                        c0 = half * 1152 + (s % 2) * 576
                        rsp = (c0 // 512 + 1) * 512
                        for r0, r1 in ((c0, rsp), (rsp, c0 + 576)):
                            nc.tensor.matmul(wpbig[:, r0:r1], h2s[:, 0, ss],
                                             w3km0[:, r0 - c0:r1 - c0],
                                             start=True, stop=False)
                            nc.tensor.matmul(wpbig[:, r0:r1], h2s[:, 1, ss],
                                             w3km1[:, r0 - c0:r1 - c0],
                                             start=False, stop=True)
                        if s % 2 == 1 or s + 1 == jc:
                            s0 = s - (s % 2)
                            nn = (s - s0 + 1) * 576
                            p0 = half * 1152
                            nc.scalar.activation(e[:, s0:s + 1, :],
                                                 wpbig[:, p0:p0 + nn], AF.Exp)
                    return go

                def mk_logit(s):
                    if not CFG["expmerge"]:
                        return mk_logit_plain(s)
                    if s % 2 == 0:
                        gpar[0] ^= 1
                    return mk_logit_merged(s, gpar[0])

                def mk_muls(j0, j1):
                    def go():
                        _lbl(f'mul[{i}]j{j0}')
                        pj0 = max(j0, min(CFG["mgrp"], j1))
                        pj1 = max(j0, min(CFG["mgrp"] + CFG["poolj"], j1))
                        for b in range(2):
                            gbf = g3[:, :, :, b, :].rearrange(
                                "p o k c -> p k (o c)").unsqueeze(2)
                            for a0, a1, eng in ((j0, pj0, nc.vector),
                                                (pj0, pj1, nc.gpsimd),
                                                (pj1, j1, nc.vector)):
                                if a0 >= a1:
                                    continue
                                gb = gbf.broadcast_to((128, 9, a1 - a0, 64))
                                eng.tensor_mul(prvf[:, :, a0:a1, b, :],
                                               gb, e5f[:, :, a0:a1, :])
                    return go

                def mk_d12(j0, j1):
                    def go():
                        _lbl(f'd12[{i}]')
                        eg = e[:, j0:j1, :]
                        nc.vector.tensor_add(eg[:, :, 0:256], eg[:, :, 0:256],
                                             eg[:, :, 256:512])
                        if CFG["d2"] == "dve":
                            nc.vector.tensor_add(eg[:, :, 0:128],
                                                 eg[:, :, 0:128],
                                                 eg[:, :, 128:256])
                    return go

                rec = (i, e, prods, jc, slot0)

                def mk_d1dma():
                    def go():
                        emit_d1dma(rec)
                    return go

                thunks = []
                j0 = 0
                for s in range(jc):
                    thunks.append(mk_logit(s))
                    if s + 1 == jc or (s + 1) % CFG['mgrp'] == 0:
                        thunks.append(mk_muls(j0, s + 1))
                        if CFG["d1"] == "dve":
                            thunks.append(mk_d12(j0, s + 1))
                        elif s + 1 == jc:
                            thunks.append(mk_d1dma())
                        thunks.append(None)  # slot for an MLP q-thunk
                        j0 = s + 1
                stage2.append(rec)
                return thunks

            def emit_d1dma(rec):
                # den level 1 on accumulating DMA: e[:,:,0:256] += e[:,:,256:512]
                i, e, prods, jc, slot0 = rec
                _lbl(f'd1dma[{i}]')
                nc.gpsimd.dma_start(e[:, :, 0:256], e[:, :, 256:512],
                                    accum_op=ADD)

            def emit_d2_dve(rec):
                i, e, prods, jc, slot0 = rec
                _lbl(f'd2[{i}]')
                nc.vector.tensor_add(e[:, :, 0:128], e[:, :, 0:128],
                                     e[:, :, 128:256])

            def emit_l2(rec):
                i, e, prods, jc, slot0 = rec
                _lbl(f'L2[{i}]')
                def ksl(a, b):
                    return prods[:, 0:jc, a:b, :].rearrange(
                        "p j k c -> p j (k c)")
                nc.gpsimd.dma_start(ksl(0, 2), ksl(2, 4), accum_op=ADD)
                if CFG["l3"] == "dma":
                    nc.gpsimd.dma_start(ksl(0, 1), ksl(1, 2), accum_op=ADD)

            def emit_l1(rec):
                i, e, prods, jc, slot0 = rec
                _lbl(f'L1[{i}]')
                def ksl(a, b):
                    return prods[:, 0:jc, a:b, :].rearrange(
                        "p j k c -> p j (k c)")
                nc.gpsimd.dma_start(ksl(0, 4), ksl(4, 8), accum_op=ADD)

            dens = {}

            def emit_d3fin(rec):
                # writes den to its own tile so e's buffer frees here (early
                # in the iteration) instead of at DVE's div at iteration end
                i, e, prods, jc, slot0 = rec
                _lbl(f'd3fin[{i}]')
                den_f = dpool.tile([128, JCAP, 64], BF16, tag="den")
                den = den_f[:, 0:jc, :]
                eng = nc.vector if CFG["d3fin"] == "dve" else nc.gpsimd
                eng.tensor_add(den, e[:, :, 0:64], e[:, :, 64:128])
                eng.tensor_add(den, den, e[:, :, 512:576])
                dens[i] = den

            def emit_acc_div(rec):
                i, e, prods, jc, slot0 = rec
                _lbl(f'accdiv[{i}]')
                k0 = prods[:, 0:jc, 0, :]
                if CFG["l3"] == "dve":
                    nc.vector.tensor_add(k0, k0, prods[:, 0:jc, 1, :])
                nc.vector.tensor_add(k0, k0, prods[:, 0:jc, 8, :])
                out_f = opool.tile([128, JCAP, 128], BF16, tag="out_t")
                out_t = out_f[:, 0:jc, :]
                ov = out_t.rearrange("p j (b c) -> p j b c", b=2)
                av = k0.rearrange("p j (b c) -> p j b c", b=2)
                den = dens.pop(i)
                rden_f = dpool.tile([128, JCAP, 64], BF16, tag="rden")
                rden = rden_f[:, 0:jc, :]
                with nc.allow_low_precision(reason="softmax denom in bf16"):
                    nc.vector.reciprocal(rden, den)
                dv = rden.unsqueeze(2).broadcast_to((128, jc, 2, 64))
                nc.vector.tensor_mul(ov, av, dv)
                nc.sync.dma_start(
                    out_d[:, slot0:slot0 + jc].rearrange("p s c -> p (s c)"),
                    out_t.rearrange("p s c -> p (s c)"))

            # pipeline per iteration i:
            #   Pool: d3fin(i-1), h2relus(i+1) (interleaved), L2(i-1), L1(i)
            #   PE:   logits(i) with MLP(i+1) q-blocks in the exp-wait gaps
            #   ACT:  exps(i) + h1relus(i+1) interleaved
            #   DVE:  mul+d12 groups(i), acc2+div(i-1)
            for th0 in mlp_thunks(0):
                th0()
            for i in range(nch):
                t = chunks[i][0]
                tn = chunks[i + 1][0] if i + 1 < nch else None
                if tn is not None and tn != t:
                    if tn + PDEPTH - 1 < nt:
                        issue_pose(tn + PDEPTH - 1)
                    if tn + GDEPTH - 1 < nt:
                        issue_gather(tn + GDEPTH - 1)
                prev = stage2.pop(0) if stage2 else None
                if prev is not None:
                    emit_d3fin(prev)
                    emit_l2(prev)
                th = stage1_thunks(i)
                mth = list(mlp_thunks(i + 1)) if i + 1 < nch else []
                mi = 0
                for thunk in th:
                    if thunk is None:
                        if mi < len(mth):
                            mth[mi]()
                            mi += 1
                    else:
                        thunk()
                while mi < len(mth):
                    mth[mi]()
                    mi += 1
                if prev is not None:
                    emit_acc_div(prev)
                if CFG["d1"] == "dma2":
                    # d2 for the CURRENT chunk at DVE's iteration end: the
                    # d1 accum-DMA (fired right after the muls) is complete
                    # by the time DVE drains its finish ops
                    emit_d2_dve(stage2[-1])
                emit_l1(stage2[-1])
                if CFG["d2"] == "pool":
                    # after L1 in Pool's queue: d2's dep (d1 on DVE) resolves
                    # later than L1's (muls), so this order avoids head-block
                    ii, ee, _pr, _jc, _sl = stage2[-1]
                    _lbl(f'd2p[{ii}]')
                    nc.gpsimd.tensor_add(ee[:, :, 0:128], ee[:, :, 0:128],
                                         ee[:, :, 128:256])
            # drain last chunk
            rec = stage2.pop(0)
            emit_d3fin(rec)
            emit_l2(rec)
            emit_acc_div(rec)

    nc.compile()
    return nc


def _schedule(interMapY, interMapX):
    Y = np.asarray(interMapY).astype(np.int64).reshape(-1)
    X = np.asarray(interMapX).astype(np.int64).reshape(-1)
    m = (Y * SCW + X).astype(np.int32)
    order = np.argsort(m, kind='stable')
    ms = m[order]
    uniq, first, inv_s, counts = np.unique(
        ms, return_index=True, return_inverse=True, return_counts=True)
    ncell = len(uniq)
    # split cells with >JCAP pixels into <=JCAP-pixel pseudo-cells (each
    # segment re-gathers the same window; gathers are cheap).  All counts
    # then fit a single chunk, tiles get near-zero occupancy spread, and
    # ragged multi-chunk tiles disappear.
    nseg = (counts + JCAP - 1) // JCAP
    segbase = np.concatenate([[0], np.cumsum(nseg)])[:-1]
    P = int(nseg.sum())
    pc_cell = np.repeat(np.arange(ncell), nseg)
    seg_idx = np.arange(P) - np.repeat(segbase, nseg)
    # balanced segment sizes (~n/nseg each, all in 9..16 for split cells):
    # avoids tiny residual segments that would form tiny-J tail tiles
    nn, kk = counts[pc_cell], nseg[pc_cell]
    counts0 = counts
    counts = nn // kk + (seg_idx < nn % kk)
    uniq = uniq[pc_cell]
    # per-pixel segment index and within-segment slot under balanced sizes
    off_px = np.arange(PXTOT) - first[inv_s]
    pN = counts0[inv_s]
    pK = (pN + JCAP - 1) // JCAP
    lo = pN // pK
    hi = pN % pK
    cut = hi * (lo + 1)
    seg_px = np.where(off_px < cut, off_px // (lo + 1),
                      hi + (off_px - cut) // np.maximum(lo, 1))
    j_px = np.where(off_px < cut, off_px % (lo + 1),
                    (off_px - cut) % np.maximum(lo, 1))
    pp_px = segbase[inv_s] + seg_px
    ncell = P
    C8 = ((ncell + 1023) // 1024) * 1024
    padn = C8 - ncell
    counts_p = np.concatenate([counts, np.zeros(padn, counts.dtype)])
    uniq_p = np.concatenate([uniq, np.zeros(padn, uniq.dtype)])
    rank_order = np.argsort(-counts_p, kind='stable')  # cellidx by rank
    R = C8
    ranks = np.arange(R)
    blk, pos = ranks // 8, ranks % 8
    core_of_rank = np.where(blk % 2 == 0, pos, 7 - pos)
    percore_pos = np.zeros(R, np.int64)
    for c in range(NCORES):
        sel = core_of_rank == c
        percore_pos[sel] = np.arange(sel.sum())
    nt = (C8 // NCORES) // 128
    # smallest-J tiles first: shorter pipeline fill at kernel start
    tile_of = nt - 1 - percore_pos // 128
    part_of = percore_pos % 128
    cnt_r = counts_p[rank_order]
    Js = np.zeros(nt, np.int64)
    for t in range(nt):
        Js[t] = max(1, cnt_r[tile_of == t].max())
    base = np.concatenate([[0], np.cumsum(Js)])[:-1]
    ST = int(Js.sum())

    rank_of_cell = np.empty(R, np.int64)
    rank_of_cell[rank_order] = ranks
    r_px = rank_of_cell[pp_px]  # per sorted-pixel (pseudo-cell rank)
    core_px = core_of_rank[r_px]
    slot_px = base[tile_of[r_px]] + j_px
    part_px = part_of[r_px]

    win = np.zeros((NCORES, nt, 128), np.int64)
    win[core_of_rank, tile_of, part_of] = uniq_p[rank_order]
    return dict(order=order, core_px=core_px, slot_px=slot_px,
                part_px=part_px, Js=tuple(int(j) for j in Js), base=base,
                ST=ST, win=win, nt=nt)


def _host_prep(x, poseMap, W1, b1, W2, b2, W3, b3, sch):
    bf = ml_dtypes.bfloat16
    xp = np.pad(np.asarray(x, np.float32), ((0, 0), (0, 0), (1, 1), (1, 1)))
    xs = np.ascontiguousarray(np.transpose(xp, (2, 3, 0, 1)))
    sw = np.lib.stride_tricks.sliding_window_view(xs, 3, axis=0)
    xsc = np.ascontiguousarray(np.transpose(sw, (0, 1, 4, 2, 3)))
    xsc = xsc.reshape(NSC * 384).astype(bf)

    ST, nt = sch["ST"], sch["nt"]
    pose_full = np.concatenate(
        [np.asarray(poseMap, np.float32)[0].reshape(3, PXTOT),
         np.ones((1, PXTOT), np.float32)], axis=0)

    w1t32 = np.zeros((4, 65), np.float32)
    w1t32[0:3, 0:64] = np.asarray(W1, np.float32).T
    w1t32[3, 0:64] = np.asarray(b1, np.float32)
    w1t32[3, 64] = 1.0
    w2t32 = np.concatenate([np.asarray(W2, np.float32).T,
                            np.asarray(b2, np.float32)[None, :]], axis=0)
    W3r = np.asarray(W3, np.float32).reshape(C, KS, KS, 256)
    w3km32 = np.ascontiguousarray(
        np.transpose(W3r, (3, 2, 1, 0))).reshape(256, 576)
    # fold b3 into... b3 is zeros in this problem; assert to be safe
    b3a = np.asarray(b3, np.float32)

    cpack = np.zeros((128, 1473), np.float32)
    cpack[:, 0:576] = w3km32[0:128]
    cpack[:, 576:1152] = w3km32[128:256]
    cpack[0:65, 1152:1408] = w2t32
    cpack[0:4, 1408:1473] = w1t32
    cpack = cpack.astype(bf)

    in_maps = []
    for c in range(NCORES):
        sel = sch["core_px"] == c
        pose = np.zeros((4, ST * 128), np.float32)
        cols = sch["slot_px"][sel] * 128 + sch["part_px"][sel]
        pose[:, cols] = pose_full[:, sch["order"][sel]]
        ids = sch["win"][c]  # [nt, 128]
        idxw = np.zeros((128, nt * 8), np.int16)
        for t in range(nt):
            a = ids[t].astype(np.int16).reshape(8, 16)  # [cc, 16]
            idxw[:, t * 8:(t + 1) * 8] = np.tile(a.T, (8, 1))
        in_maps.append({
            "xw": xsc,
            "idx": np.ascontiguousarray(idxw),
            "pose": np.ascontiguousarray(pose).astype(bf),
            "cpack": cpack,
        })
    return in_maps, b3a


def kernel(**inputs):
    global LAST_RESULT, LAST_PROG
    sch = _schedule(inputs["interMapY"], inputs["interMapX"])
    key = sch["Js"]
    if key not in _PROG_CACHE:
        _PROG_CACHE[key] = build_program(sch["Js"])
    nc = _PROG_CACHE[key]
    LAST_PROG = nc
    in_maps, b3a = _host_prep(
        inputs["x"], inputs["poseMap"], inputs["W1"], inputs["b1"],
        inputs["W2"], inputs["b2"], inputs["W3"], inputs["b3"], sch)
    os.environ.setdefault("BASS_NEVER_TRACE", "1")
    res = None
    last_err = None
    for attempt in range(3):
        try:
            res = run_bass_kernel_spmd(nc, in_maps, list(range(NCORES)))
            break
        except Exception as err:
            last_err = err
            os.environ["NEURON_RT_RESET_CORES"] = "1"
    if res is None:
        raise last_err
    LAST_RESULT = res
    ST = sch["ST"]
    out_full = np.zeros((PXTOT, 128), np.float32)
    for c in range(NCORES):
        arr = np.asarray(res.results[c]["out"]).reshape(128, ST, 128)
        sel = sch["core_px"] == c
        out_full[sch["order"][sel]] = arr[
            sch["part_px"][sel], sch["slot_px"][sel], :].astype(np.float32)
    out = out_full.reshape(HO, WO, BS, C).transpose(2, 3, 0, 1)
    # b3 correction: b3 is zero in this problem's setup; softmax with b3
    # would change weights -- recompute would be needed.  Guard loudly.
    if np.any(b3a != 0.0):
        raise NotImplementedError("nonzero b3 not folded in this kernel")
    return np.ascontiguousarray(out)


if __name__ == "__main__":
    data = np.load(sys.argv[1] if len(sys.argv) > 1 else "work/inputs.npz")
    out = kernel(**{k: data[k] for k in data.files})
    print("out", out.shape, out.dtype, float(np.abs(out).max()))


# revision 57
# speedup vs baseline: 1.0437x; 1.0040x over previous
"""Trainium2 Bass kernel for nn_BilinearUpsampler (cell-sorted v3).

out[b,c,i,j] = sum_k softmax_k(MLP(poseMap)[c,k,i,j]) * xpad[b,c,Y[i,j]+dy_k,X[i,j]+dx_k]

Key idea vs the per-pixel-gather baseline: output pixels are grouped by
their source cell (Y,X).  A tile = 128 cells (on partitions) x J slots
(pixels of that cell, on the free axis).  The 3x3 input window (1152
bf16 = one "cell window") is gathered ONCE per cell (128 descriptors
per tile instead of one per pixel -> 16x less DMA) and broadcast over
the cell's pixels with a stride-0 AP in the tap multiply.  Cells are
occupancy-sorted into tiles so J per tile ~= every cell's count (pad
waste ~2-4%).  The schedule is data-dependent; the program is compiled
per schedule signature and cached.

Work placement (tuned against the TimelineSim cost model; PSUM reads
are only legal on PE/ACT/DVE, so all PSUM evictions sit on ACT):
  PE   : MLP (3->64->256) + logits (256->576)
  ACT  : h1/h2 relu evictions, exp eviction (PSUM->SBUF)
  DVE  : tap muls (bf16 2x, g broadcast over j via stride-0 AP), den
         d1/d2 (in-place on e), tree finish (k0+=k1, k0+=k8 in prods),
         reciprocal, final normalize multiply
  Pool : den d3/final into a separate den tile (frees e early -- this
         breaks the exp->...->div buffer cycle that serialized ACT
         behind DVE), SWDGE descriptor-gen for gathers and accum DMAs
  DMA  : per-cell window gather, accumulating SBUF->SBUF tap-tree
         levels L1 (k[0:4]+=k[4:8]) and L2 (k[0:2]+=k[2:4]), pose in,
         out store

Pipeline per iteration i (in-order engine queues head-block on the
first not-ready instruction, so per-engine emission order is chosen to
match dependency readiness): d2+den(i-1) -> logits/exp(i) subtiles with
tap-mul groups spliced in and MLP(i+1) q-blocks in the exp-wait gaps ->
L2(i-1) -> tree-finish+recip+normalize+store(i-1) -> L1(i).
"""

import sys
import os

sys.path.insert(0, "/opt/trn_rl_repo")

import numpy as np
import ml_dtypes

import concourse.bass as bass
import concourse.bacc as bacc
import concourse.mybir as mybir
import concourse.tile as tile
import concourse.alu_op_type as alu
from concourse.bass_utils import run_bass_kernel_spmd

BF16 = mybir.dt.bfloat16
F32 = mybir.dt.float32
I16 = mybir.dt.int16
AF = mybir.ActivationFunctionType
ADD = alu.AluOpType.add

NCORES = 8
C = 64
KS = 3
BS = 2
HI = WI = 128
HO = WO = 512
SCW = HI + 2  # supercell grid width (padded x: 0..129)
NSC = HI * SCW  # supercells
NWIN = NSC - 2  # gatherable 3-supercell windows
PXTOT = HO * WO

JCAP = 16  # max j-slots per chunk

# work-placement knobs (swept against the cost model)
CFG = {
    "d1": "dve",      # den level 1: 'dve' | 'dma'
    "d3fin": "pool",   # den level 3 + final: 'dve' | 'pool'
    "h2relu": "act",  # h2 relu eviction: 'act' (pool cannot read PSUM)
    "poolj": 0,       # j-slots per chunk whose tap-muls run on Pool
    "mgrp": 8,        # j-slots per tap-mul group
}

LAST_RESULT = None
LAST_PROG = None

_PROG_CACHE = {}

LABEL_HOOK = None


def _lbl(s):
    if LABEL_HOOK is not None:
        LABEL_HOOK(s)


def _chunks(J):
    nch = (J + JCAP - 1) // JCAP
    lo = J // nch
    hi = lo + 1
    nhi = J - lo * nch
    return [hi] * nhi + [lo] * (nch - nhi)


def build_program(Js):
    """Js: per-tile slot counts (shared across all 8 cores)."""
    Js = list(Js)
    nt = len(Js)
    ST = sum(Js)
    base = np.concatenate([[0], np.cumsum(Js)])[:-1]
    JMAXT = max(Js)

    nc = bacc.Bacc("TRN2", target_bir_lowering=False, debug=False,
                   num_devices=NCORES)

    xw_d = nc.dram_tensor("xw", [NSC * 384], BF16, kind="ExternalInput")
    idx_d = nc.dram_tensor("idx", [128, nt * 8], I16, kind="ExternalInput")
    pose_d = nc.dram_tensor("pose", [4, ST * 128], BF16, kind="ExternalInput")
    # cpack: [0:576] w3km0, [576:1152] w3km1, [1152:1408] w2t (rows 0-64),
    # [1408:1473] w1t (rows 0-3)
    cpack_d = nc.dram_tensor("cpack", [128, 1473], BF16, kind="ExternalInput")
    out_d = nc.dram_tensor("out", [128, ST, 128], BF16, kind="ExternalOutput")

    import bass_rust

    def x_windows_ap():
        ap = xw_d[:].copy()
        ap.ap = bass_rust.VecI64Pair([(384, NWIN), (1, 1152)])
        return ap

    # flat chunk list: (tile, jb, jc, slot0)
    chunks = []
    for t in range(nt):
        jb = 0
        for jc in _chunks(Js[t]):
            chunks.append((t, jb, jc, int(base[t]) + jb))
            jb += jc
    nch = len(chunks)

    with tile.TileContext(nc) as tc:
        with (
            tc.tile_pool(name="consts", bufs=1) as cpool,
            tc.tile_pool(name="pose", bufs=3) as ppool,
            tc.tile_pool(name="gath", bufs=3) as gpool,
            tc.tile_pool(name="mlp", bufs=2) as mpool,
            tc.tile_pool(name="ework", bufs=2) as epool,
            tc.tile_pool(name="prod", bufs=2) as vpool,
            tc.tile_pool(name="dpool", bufs=2) as dpool,
            tc.tile_pool(name="outp", bufs=2) as opool,
            tc.tile_pool(name="ph1", bufs=1 if CFG["expmerge"] else 2,
                         space="PSUM") as ph1,
            tc.tile_pool(name="ph2", bufs=1, space="PSUM") as ph2,
            tc.tile_pool(name="pw", bufs=1 if CFG["expmerge"] else 2,
                         space="PSUM") as pw,
        ):
            # ---- constants ----
            # persistent logits slab for expmerge: two 1152-col halves,
            # alternated globally; each half holds a PAIR of subtiles so
            # exp runs once per pair (halves ACT per-op overhead)
            if CFG["expmerge"]:
                wpbig = pw.tile([128, 2304], F32, tag="wp")
            else:
                wpbig = None
            gpar = [0]  # global pair parity
            idxt = cpool.tile([128, nt * 8], I16, tag="idxt")
            nc.sync.dma_start(idxt[:], idx_d[:])
            cp = cpool.tile([128, 1473], BF16, tag="cpack")
            nc.sync.dma_start(cp[:], cpack_d[:])
            w3km0 = cp[:, 0:576]
            w3km1 = cp[:, 576:1152]
            w2t = cp[0:65, 1152:1408]
            w1t = cp[0:4, 1408:1473]

            xwin = x_windows_ap()

            GDEPTH = 2
            PDEPTH = 2
            gtiles = {}
            ptiles = {}

            def issue_pose(t):
                J = Js[t]
                _lbl(f'pose[{t}]')
                p3 = ppool.tile([4, JMAXT * 128], BF16, tag="p3")
                nc.sync.dma_start(
                    p3[:, 0:J * 128],
                    pose_d[:, base[t] * 128:(base[t] + J) * 128])
                ptiles[t] = p3

            def issue_gather(t):
                _lbl(f'gather[{t}]')
                g = gpool.tile([128, 1, 1152], BF16, tag="g")
                nc.gpsimd.dma_gather(
                    out_ap=g[:],
                    in_ap=xwin,
                    idxs_ap=idxt[:, t * 8:(t + 1) * 8],
                    num_idxs=128,
                    num_idxs_reg=128,
                    elem_size=1152,
                    elem_step=384,
                )
                gtiles[t] = g

            for t in range(min(PDEPTH, nt)):
                issue_pose(t)
            for t in range(min(GDEPTH, nt)):
                issue_gather(t)

            mlp_done = {}  # chunk idx -> (h1s, h2s)

            def mlp_thunks(i):
                """Per-q-chunk MLP thunks for chunk i (h1mm, h1relu, h2mms,
                h2relu); interleaved into the previous chunk's stage-1
                stream so evictions are ready early."""
                t, jb, jc, _ = chunks[i]
                p3 = ptiles[t]
                npx = jc * 128
                h1s = mpool.tile([65, JCAP * 128], BF16, tag="h1s")
                h2s = mpool.tile([128, 2, JCAP * 128], BF16, tag="h2s")
                mlp_done[i] = (h1s, h2s)

                def mk(q0):
                    def go():
                        qn = min(512, npx - q0)
                        qs = slice(jb * 128 + q0, jb * 128 + q0 + qn)
                        qd = slice(q0, q0 + qn)
                        _lbl(f'mlp[{i}]q{q0//512}')
                        h1p = ph1.tile([65, 512], F32, tag="h1p")
                        nc.tensor.matmul(h1p[:, 0:qn], w1t, p3[:, qs],
                                         start=True, stop=True)
                        nc.scalar.activation(h1s[:, qd], h1p[:, 0:qn],
                                             AF.Relu)
                        h2p = ph2.tile([128, 2, 512], F32, tag="h2p")
                        for cc in range(2):
                            nc.tensor.matmul(h2p[:, cc, 0:qn],
                                             w2t[:, cc * 128:(cc + 1) * 128],
                                             h1s[:, qd], start=True,
                                             stop=True)
                        m = CFG["h2relu"]
                        on_act = m == "act" or (m == "alt" and
                                                (q0 // 512) % 2 == 1)
                        if on_act:
                            nc.scalar.activation(h2s[:, :, qd],
                                                 h2p[:, :, 0:qn], AF.Relu)
                        else:
                            nc.gpsimd.tensor_scalar_max(h2s[:, :, qd],
                                                        h2p[:, :, 0:qn], 0.0)
                    return go

                return [mk(q0) for q0 in range(0, npx, 512)]

            stage2 = []

            def stage1_thunks(i):
                """Thunks for chunk i: per-subtile logits+exp, mul groups
                after every 4 exps, then d1/d2.  Appends to stage2."""
                t, jb, jc, slot0 = chunks[i]
                h1s, h2s = mlp_done.pop(i)
                g = gtiles[t]
                e_f = epool.tile([128, JCAP, 576], BF16, tag="e_t")
                e = e_f[:, 0:jc, :]
                g3 = g.rearrange("p o (k b c) -> p o k b c", k=9, b=2)
                prods = vpool.tile([128, JCAP, 9, 128], BF16, tag="prods")
                e5f = e.rearrange("p j (k c) -> p k j c", k=9)
                prvf = prods[:, 0:jc, :, :].rearrange(
                    "p j k (b c) -> p k j b c", b=2)

                def mk_logit_plain(s):
                    def go():
                        _lbl(f'logit[{i}]s{s}')
                        ss = slice(s * 128, s * 128 + 128)
                        wp = pw.tile([128, 576], F32, tag="wp")
                        for r0, r1 in ((0, 512), (512, 576)):
                            nc.tensor.matmul(wp[:, r0:r1], h2s[:, 0, ss],
                                             w3km0[:, r0:r1], start=True,
                                             stop=False)
                            nc.tensor.matmul(wp[:, r0:r1], h2s[:, 1, ss],
                                             w3km1[:, r0:r1], start=False,
                                             stop=True)
                        nc.scalar.activation(e[:, s, :], wp[:], AF.Exp)
                    return go

                def mk_logit_merged(s, half):
                    def go():
                        _lbl(f'logit[{i}]s{s}')
                        ss = slice(s * 128, s * 128 + 128)
                        c0 = half * 1152 + (s % 2) * 576
                        rsp = (c0 // 512 + 1) * 512
                        for r0, r1 in ((c0, rsp), (rsp, c0 + 576)):
                            nc.tensor.matmul(wpbig[:, r0:r1], h2s[:, 0, ss],
                                             w3km0[:, r0 - c0:r1 - c0],
                                             start=True, stop=False)
                            nc.tensor.matmul(wpbig[:, r0:r1], h2s[:, 1, ss],
                                             w3km1[:, r0 - c0:r1 - c0],
                                             start=False, stop=True)
                        if s % 2 == 1 or s + 1 == jc:
                            s0 = s - (s % 2)
                            nn = (s - s0 + 1) * 576
                            p0 = half * 1152
                            nc.scalar.activation(e[:, s0:s + 1, :],
                                                 wpbig[:, p0:p0 + nn], AF.Exp)
                    return go

                def mk_logit(s):
                    if not CFG["expmerge"]:
                        return mk_logit_plain(s)
                    if s % 2 == 0:
                        gpar[0] ^= 1
                    return mk_logit_merged(s, gpar[0])

                def mk_muls(j0, j1):
                    def go():
                        _lbl(f'mul[{i}]j{j0}')
                        pj0 = max(j0, min(CFG["mgrp"], j1))
                        pj1 = max(j0, min(CFG["mgrp"] + CFG["poolj"], j1))
                        for b in range(2):
                            gbf = g3[:, :, :, b, :].rearrange(
                                "p o k c -> p k (o c)").unsqueeze(2)
                            for a0, a1, eng in ((j0, pj0, nc.vector),
                                                (pj0, pj1, nc.gpsimd),
                                                (pj1, j1, nc.vector)):
                                if a0 >= a1:
                                    continue
                                gb = gbf.broadcast_to((128, 9, a1 - a0, 64))
                                eng.tensor_mul(prvf[:, :, a0:a1, b, :],
                                               gb, e5f[:, :, a0:a1, :])
                    return go

                def mk_d12(j0, j1):
                    def go():
                        _lbl(f'd12[{i}]')
                        eg = e[:, j0:j1, :]
                        nc.vector.tensor_add(eg[:, :, 0:256], eg[:, :, 0:256],
                                             eg[:, :, 256:512])
                        if CFG["d2"] == "dve":
                            nc.vector.tensor_add(eg[:, :, 0:128],
                                                 eg[:, :, 0:128],
                                                 eg[:, :, 128:256])
                    return go

                rec = (i, e, prods, jc, slot0)

                def mk_d1dma():
                    def go():
                        emit_d1dma(rec)
                    return go

                thunks = []
                j0 = 0
                for s in range(jc):
                    thunks.append(mk_logit(s))
                    if s + 1 == jc or (s + 1) % CFG['mgrp'] == 0:
                        thunks.append(mk_muls(j0, s + 1))
                        if CFG["d1"] == "dve":
                            thunks.append(mk_d12(j0, s + 1))
                        elif s + 1 == jc:
                            thunks.append(mk_d1dma())
                        thunks.append(None)  # slot for an MLP q-thunk
                        j0 = s + 1
                stage2.append(rec)
                return thunks

            def emit_d1dma(rec):
                # den level 1 on accumulating DMA: e[:,:,0:256] += e[:,:,256:512]
                i, e, prods, jc, slot0 = rec
                _lbl(f'd1dma[{i}]')
                nc.gpsimd.dma_start(e[:, :, 0:256], e[:, :, 256:512],
                                    accum_op=ADD)

            def emit_d2_dve(rec):
                i, e, prods, jc, slot0 = rec
                _lbl(f'd2[{i}]')
                nc.vector.tensor_add(e[:, :, 0:128], e[:, :, 0:128],
                                     e[:, :, 128:256])

            def emit_l2(rec):
                i, e, prods, jc, slot0 = rec
                _lbl(f'L2[{i}]')
                def ksl(a, b):
                    return prods[:, 0:jc, a:b, :].rearrange(
                        "p j k c -> p j (k c)")
                nc.gpsimd.dma_start(ksl(0, 2), ksl(2, 4), accum_op=ADD)
                if CFG["l3"] == "dma":
                    nc.gpsimd.dma_start(ksl(0, 1), ksl(1, 2), accum_op=ADD)

            def emit_l1(rec):
                i, e, prods, jc, slot0 = rec
                _lbl(f'L1[{i}]')
                def ksl(a, b):
                    return prods[:, 0:jc, a:b, :].rearrange(
                        "p j k c -> p j (k c)")
                nc.gpsimd.dma_start(ksl(0, 4), ksl(4, 8), accum_op=ADD)

            dens = {}

            def emit_d3fin(rec):
                # writes den to its own tile so e's buffer frees here (early
                # in the iteration) instead of at DVE's div at iteration end
                i, e, prods, jc, slot0 = rec
                _lbl(f'd3fin[{i}]')
                den_f = dpool.tile([128, JCAP, 64], BF16, tag="den")
                den = den_f[:, 0:jc, :]
                eng = nc.vector if CFG["d3fin"] == "dve" else nc.gpsimd
                eng.tensor_add(den, e[:, :, 0:64], e[:, :, 64:128])
                eng.tensor_add(den, den, e[:, :, 512:576])
                dens[i] = den

            def emit_acc_div(rec):
                i, e, prods, jc, slot0 = rec
                _lbl(f'accdiv[{i}]')
                k0 = prods[:, 0:jc, 0, :]
                if CFG["l3"] == "dve":
                    nc.vector.tensor_add(k0, k0, prods[:, 0:jc, 1, :])
                nc.vector.tensor_add(k0, k0, prods[:, 0:jc, 8, :])
                out_f = opool.tile([128, JCAP, 128], BF16, tag="out_t")
                out_t = out_f[:, 0:jc, :]
                ov = out_t.rearrange("p j (b c) -> p j b c", b=2)
                av = k0.rearrange("p j (b c) -> p j b c", b=2)
                den = dens.pop(i)
                rden_f = dpool.tile([128, JCAP, 64], BF16, tag="rden")
                rden = rden_f[:, 0:jc, :]
                with nc.allow_low_precision(reason="softmax denom in bf16"):
                    nc.vector.reciprocal(rden, den)
                dv = rden.unsqueeze(2).broadcast_to((128, jc, 2, 64))
                nc.vector.tensor_mul(ov, av, dv)
                nc.sync.dma_start(
                    out_d[:, slot0:slot0 + jc].rearrange("p s c -> p (s c)"),
                    out_t.rearrange("p s c -> p (s c)"))

            # pipeline per iteration i:
            #   Pool: d3fin(i-1), h2relus(i+1) (interleaved), L2(i-1), L1(i)
            #   PE:   logits(i) with MLP(i+1) q-blocks in the exp-wait gaps
            #   ACT:  exps(i) + h1relus(i+1) interleaved
            #   DVE:  mul+d12 groups(i), acc2+div(i-1)
            for th0 in mlp_thunks(0):
                th0()
            for i in range(nch):
                t = chunks[i][0]
                tn = chunks[i + 1][0] if i + 1 < nch else None
                if tn is not None and tn != t:
                    if tn + PDEPTH - 1 < nt:
                        issue_pose(tn + PDEPTH - 1)
                    if tn + GDEPTH - 1 < nt:
                        issue_gather(tn + GDEPTH - 1)
                prev = stage2.pop(0) if stage2 else None
                if prev is not None:
                    emit_d3fin(prev)
                    emit_l2(prev)
                th = stage1_thunks(i)
                mth = list(mlp_thunks(i + 1)) if i + 1 < nch else []
                mi = 0
                for thunk in th:
                    if thunk is None:
                        if mi < len(mth):
                            mth[mi]()
                            mi += 1
                    else:
                        thunk()
                while mi < len(mth):
                    mth[mi]()
                    mi += 1
                if prev is not None:
                    emit_acc_div(prev)
                if CFG["d1"] == "dma2":
                    # d2 for the CURRENT chunk at DVE's iteration end: the
                    # d1 accum-DMA (fired right after the muls) is complete
                    # by the time DVE drains its finish ops
                    emit_d2_dve(stage2[-1])
                emit_l1(stage2[-1])
                if CFG["d2"] == "pool":
                    # after L1 in Pool's queue: d2's dep (d1 on DVE) resolves
                    # later than L1's (muls), so this order avoids head-block
                    ii, ee, _pr, _jc, _sl = stage2[-1]
                    _lbl(f'd2p[{ii}]')
                    nc.gpsimd.tensor_add(ee[:, :, 0:128], ee[:, :, 0:128],
                                         ee[:, :, 128:256])
            # drain last chunk
            rec = stage2.pop(0)
            emit_d3fin(rec)
            emit_l2(rec)
            emit_acc_div(rec)

    nc.compile()
    return nc


def _schedule(interMapY, interMapX):
    Y = np.asarray(interMapY).astype(np.int64).reshape(-1)
    X = np.asarray(interMapX).astype(np.int64).reshape(-1)
    m = (Y * SCW + X).astype(np.int32)
    order = np.argsort(m, kind='stable')
    ms = m[order]
    uniq, first, inv_s, counts = np.unique(
        ms, return_index=True, return_inverse=True, return_counts=True)
    ncell = len(uniq)
    # split cells with >JCAP pixels into <=JCAP-pixel pseudo-cells (each
    # segment re-gathers the same window; gathers are cheap).  All counts
    # then fit a single chunk, tiles get near-zero occupancy spread, and
    # ragged multi-chunk tiles disappear.
    nseg = (counts + JCAP - 1) // JCAP
    segbase = np.concatenate([[0], np.cumsum(nseg)])[:-1]
    P = int(nseg.sum())
    pc_cell = np.repeat(np.arange(ncell), nseg)
    seg_idx = np.arange(P) - np.repeat(segbase, nseg)
    # balanced segment sizes (~n/nseg each, all in 9..16 for split cells):
    # avoids tiny residual segments that would form tiny-J tail tiles
    nn, kk = counts[pc_cell], nseg[pc_cell]
    counts0 = counts
    counts = nn // kk + (seg_idx < nn % kk)
    uniq = uniq[pc_cell]
    # per-pixel segment index and within-segment slot under balanced sizes
    off_px = np.arange(PXTOT) - first[inv_s]
    pN = counts0[inv_s]
    pK = (pN + JCAP - 1) // JCAP
    lo = pN // pK
    hi = pN % pK
    cut = hi * (lo + 1)
    seg_px = np.where(off_px < cut, off_px // (lo + 1),
                      hi + (off_px - cut) // np.maximum(lo, 1))
    j_px = np.where(off_px < cut, off_px % (lo + 1),
                    (off_px - cut) % np.maximum(lo, 1))
    pp_px = segbase[inv_s] + seg_px
    ncell = P
    C8 = ((ncell + 1023) // 1024) * 1024
    padn = C8 - ncell
    counts_p = np.concatenate([counts, np.zeros(padn, counts.dtype)])
    uniq_p = np.concatenate([uniq, np.zeros(padn, uniq.dtype)])
    rank_order = np.argsort(-counts_p, kind='stable')  # cellidx by rank
    R = C8
    ranks = np.arange(R)
    blk, pos = ranks // 8, ranks % 8
    core_of_rank = np.where(blk % 2 == 0, pos, 7 - pos)
    percore_pos = np.zeros(R, np.int64)
    for c in range(NCORES):
        sel = core_of_rank == c
        percore_pos[sel] = np.arange(sel.sum())
    nt = (C8 // NCORES) // 128
    # pyramid tile order: small-J tiles at both ends (short pipeline fill
    # AND short drain), largest in the middle
    blk = percore_pos // 128
    asc = np.arange(nt - 1, -1, -1)  # blocks by J ascending
    schd = np.concatenate([asc[0::2], asc[1::2][::-1]])
    pos_of_block = np.empty(nt, np.int64)
    pos_of_block[schd] = np.arange(nt)
    tile_of = pos_of_block[blk]
    part_of = percore_pos % 128
    cnt_r = counts_p[rank_order]
    Js = np.zeros(nt, np.int64)
    for t in range(nt):
        Js[t] = max(1, cnt_r[tile_of == t].max())
    base = np.concatenate([[0], np.cumsum(Js)])[:-1]
    ST = int(Js.sum())

    rank_of_cell = np.empty(R, np.int64)
    rank_of_cell[rank_order] = ranks
    r_px = rank_of_cell[pp_px]  # per sorted-pixel (pseudo-cell rank)
    core_px = core_of_rank[r_px]
    slot_px = base[tile_of[r_px]] + j_px
    part_px = part_of[r_px]

    win = np.zeros((NCORES, nt, 128), np.int64)
    win[core_of_rank, tile_of, part_of] = uniq_p[rank_order]
    return dict(order=order, core_px=core_px, slot_px=slot_px,
                part_px=part_px, Js=tuple(int(j) for j in Js), base=base,
                ST=ST, win=win, nt=nt)


def _host_prep(x, poseMap, W1, b1, W2, b2, W3, b3, sch):
    bf = ml_dtypes.bfloat16
    xp = np.pad(np.asarray(x, np.float32), ((0, 0), (0, 0), (1, 1), (1, 1)))
    xs = np.ascontiguousarray(np.transpose(xp, (2, 3, 0, 1)))
    sw = np.lib.stride_tricks.sliding_window_view(xs, 3, axis=0)
    xsc = np.ascontiguousarray(np.transpose(sw, (0, 1, 4, 2, 3)))
    xsc = xsc.reshape(NSC * 384).astype(bf)

    ST, nt = sch["ST"], sch["nt"]
    pose_full = np.concatenate(
        [np.asarray(poseMap, np.float32)[0].reshape(3, PXTOT),
         np.ones((1, PXTOT), np.float32)], axis=0)

    w1t32 = np.zeros((4, 65), np.float32)
    w1t32[0:3, 0:64] = np.asarray(W1, np.float32).T
    w1t32[3, 0:64] = np.asarray(b1, np.float32)
    w1t32[3, 64] = 1.0
    w2t32 = np.concatenate([np.asarray(W2, np.float32).T,
                            np.asarray(b2, np.float32)[None, :]], axis=0)
    W3r = np.asarray(W3, np.float32).reshape(C, KS, KS, 256)
    w3km32 = np.ascontiguousarray(
        np.transpose(W3r, (3, 2, 1, 0))).reshape(256, 576)
    # fold b3 into... b3 is zeros in this problem; assert to be safe
    b3a = np.asarray(b3, np.float32)

    cpack = np.zeros((128, 1473), np.float32)
    cpack[:, 0:576] = w3km32[0:128]
    cpack[:, 576:1152] = w3km32[128:256]
    cpack[0:65, 1152:1408] = w2t32
    cpack[0:4, 1408:1473] = w1t32
    cpack = cpack.astype(bf)

    in_maps = []
    for c in range(NCORES):
        sel = sch["core_px"] == c
        pose = np.zeros((4, ST * 128), np.float32)
        cols = sch["slot_px"][sel] * 128 + sch["part_px"][sel]
        pose[:, cols] = pose_full[:, sch["order"][sel]]
        ids = sch["win"][c]  # [nt, 128]
        idxw = np.zeros((128, nt * 8), np.int16)
        for t in range(nt):
            a = ids[t].astype(np.int16).reshape(8, 16)  # [cc, 16]
            idxw[:, t * 8:(t + 1) * 8] = np.tile(a.T, (8, 1))
        in_maps.append({
            "xw": xsc,
            "idx": np.ascontiguousarray(idxw),
            "pose": np.ascontiguousarray(pose).astype(bf),
            "cpack": cpack,
        })
    return in_maps, b3a


def kernel(**inputs):
    global LAST_RESULT, LAST_PROG
    sch = _schedule(inputs["interMapY"], inputs["interMapX"])
    key = sch["Js"]
    if key not in _PROG_CACHE:
        _PROG_CACHE[key] = build_program(sch["Js"])
    nc = _PROG_CACHE[key]
    LAST_PROG = nc
    in_maps, b3a = _host_prep(
        inputs["x"], inputs["poseMap"], inputs["W1"], inputs["b1"],
        inputs["W2"], inputs["b2"], inputs["W3"], inputs["b3"], sch)
    os.environ.setdefault("BASS_NEVER_TRACE", "1")
    res = None
    last_err = None
    for attempt in range(3):
        try:
            res = run_bass_kernel_spmd(nc, in_maps, list(range(NCORES)))
            break
        except Exception as err:
            last_err = err
            os.environ["NEURON_RT_RESET_CORES"] = "1"
    if res is None:
        raise last_err
    LAST_RESULT = res
    ST = sch["ST"]
    out_full = np.zeros((PXTOT, 128), np.float32)
    for c in range(NCORES):
        arr = np.asarray(res.results[c]["out"]).reshape(128, ST, 128)
        sel = sch["core_px"] == c
        out_full[sch["order"][sel]] = arr[
            sch["part_px"][sel], sch["slot_px"][sel], :].astype(np.float32)
    out = out_full.reshape(HO, WO, BS, C).transpose(2, 3, 0, 1)
    # b3 correction: b3 is zero in this problem's setup; softmax with b3
    # would change weights -- recompute would be needed.  Guard loudly.
    if np.any(b3a != 0.0):
        raise NotImplementedError("nonzero b3 not folded in this kernel")
    return np.ascontiguousarray(out)


if __name__ == "__main__":
    data = np.load(sys.argv[1] if len(sys.argv) > 1 else "work/inputs.npz")
    out = kernel(**{k: data[k] for k in data.files})
    print("out", out.shape, out.dtype, float(np.abs(out).max()))


# revision 58
# speedup vs baseline: 1.0732x; 1.0283x over previous
"""Trainium2 Bass kernel for nn_BilinearUpsampler (cell-sorted v3).

out[b,c,i,j] = sum_k softmax_k(MLP(poseMap)[c,k,i,j]) * xpad[b,c,Y[i,j]+dy_k,X[i,j]+dx_k]

Key idea vs the per-pixel-gather baseline: output pixels are grouped by
their source cell (Y,X).  A tile = 128 cells (on partitions) x J slots
(pixels of that cell, on the free axis).  The 3x3 input window (1152
bf16 = one "cell window") is gathered ONCE per cell (128 descriptors
per tile instead of one per pixel -> 16x less DMA) and broadcast over
the cell's pixels with a stride-0 AP in the tap multiply.  Cells are
occupancy-sorted into tiles so J per tile ~= every cell's count (pad
waste ~2-4%).  The schedule is data-dependent; the program is compiled
per schedule signature and cached.

Work placement (tuned against the TimelineSim cost model; PSUM reads
are only legal on PE/ACT/DVE, so all PSUM evictions sit on ACT):
  PE   : MLP (3->64->256) + logits (256->576)
  ACT  : h1/h2 relu evictions, exp eviction (PSUM->SBUF)
  DVE  : tap muls (bf16 2x, g broadcast over j via stride-0 AP), den
         d1/d2 (in-place on e), tree finish (k0+=k1, k0+=k8 in prods),
         reciprocal, final normalize multiply
  Pool : den d3/final into a separate den tile (frees e early -- this
         breaks the exp->...->div buffer cycle that serialized ACT
         behind DVE), SWDGE descriptor-gen for gathers and accum DMAs
  DMA  : per-cell window gather, accumulating SBUF->SBUF tap-tree
         levels L1 (k[0:4]+=k[4:8]) and L2 (k[0:2]+=k[2:4]), pose in,
         out store

Pipeline per iteration i (in-order engine queues head-block on the
first not-ready instruction, so per-engine emission order is chosen to
match dependency readiness): d2+den(i-1) -> logits/exp(i) subtiles with
tap-mul groups spliced in and MLP(i+1) q-blocks in the exp-wait gaps ->
L2(i-1) -> tree-finish+recip+normalize+store(i-1) -> L1(i).
"""

import sys
import os

sys.path.insert(0, "/opt/trn_rl_repo")

import numpy as np
import ml_dtypes

import concourse.bass as bass
import concourse.bacc as bacc
import concourse.mybir as mybir
import concourse.tile as tile
import concourse.alu_op_type as alu
from concourse.bass_utils import run_bass_kernel_spmd

BF16 = mybir.dt.bfloat16
F32 = mybir.dt.float32
I16 = mybir.dt.int16
AF = mybir.ActivationFunctionType
ADD = alu.AluOpType.add

NCORES = 8
C = 64
KS = 3
BS = 2
HI = WI = 128
HO = WO = 512
SCW = HI + 2  # supercell grid width (padded x: 0..129)
NSC = HI * SCW  # supercells
NWIN = NSC - 2  # gatherable 3-supercell windows
PXTOT = HO * WO

JCAP = 16  # max j-slots per chunk

# work-placement knobs (swept against the cost model)
CFG = {
    "d1": "dve",      # den level 1: 'dve' | 'dma'
    "d3fin": "pool",   # den level 3 + final: 'dve' | 'pool'
    "h2relu": "act",  # h2 relu eviction: 'act' (pool cannot read PSUM)
    "poolj": 1,       # j-slots per chunk whose tap-muls run on Pool
    "mgrp": 8,        # j-slots per tap-mul group
}

LAST_RESULT = None
LAST_PROG = None

_PROG_CACHE = {}

LABEL_HOOK = None


def _lbl(s):
    if LABEL_HOOK is not None:
        LABEL_HOOK(s)


def _chunks(J):
    nch = (J + JCAP - 1) // JCAP
    lo = J // nch
    hi = lo + 1
    nhi = J - lo * nch
    return [hi] * nhi + [lo] * (nch - nhi)


def build_program(Js):
    """Js: per-tile slot counts (shared across all 8 cores)."""
    Js = list(Js)
    nt = len(Js)
    ST = sum(Js)
    base = np.concatenate([[0], np.cumsum(Js)])[:-1]
    JMAXT = max(Js)

    nc = bacc.Bacc("TRN2", target_bir_lowering=False, debug=False,
                   num_devices=NCORES)

    xw_d = nc.dram_tensor("xw", [NSC * 384], BF16, kind="ExternalInput")
    idx_d = nc.dram_tensor("idx", [128, nt * 8], I16, kind="ExternalInput")
    pose_d = nc.dram_tensor("pose", [4, ST * 128], BF16, kind="ExternalInput")
    # cpack: [0:576] w3km0, [576:1152] w3km1, [1152:1408] w2t (rows 0-64),
    # [1408:1473] w1t (rows 0-3)
    cpack_d = nc.dram_tensor("cpack", [128, 1473], BF16, kind="ExternalInput")
    out_d = nc.dram_tensor("out", [128, ST, 128], BF16, kind="ExternalOutput")

    import bass_rust

    def x_windows_ap():
        ap = xw_d[:].copy()
        ap.ap = bass_rust.VecI64Pair([(384, NWIN), (1, 1152)])
        return ap

    # flat chunk list: (tile, jb, jc, slot0)
    chunks = []
    for t in range(nt):
        jb = 0
        for jc in _chunks(Js[t]):
            chunks.append((t, jb, jc, int(base[t]) + jb))
            jb += jc
    nch = len(chunks)

    with tile.TileContext(nc) as tc:
        with (
            tc.tile_pool(name="consts", bufs=1) as cpool,
            tc.tile_pool(name="pose", bufs=3) as ppool,
            tc.tile_pool(name="gath", bufs=3) as gpool,
            tc.tile_pool(name="mlp", bufs=2) as mpool,
            tc.tile_pool(name="ework", bufs=2) as epool,
            tc.tile_pool(name="prod", bufs=2) as vpool,
            tc.tile_pool(name="dpool", bufs=2) as dpool,
            tc.tile_pool(name="outp", bufs=2) as opool,
            tc.tile_pool(name="ph1", bufs=1 if CFG["expmerge"] else 2,
                         space="PSUM") as ph1,
            tc.tile_pool(name="ph2", bufs=1, space="PSUM") as ph2,
            tc.tile_pool(name="pw", bufs=1 if CFG["expmerge"] else 2,
                         space="PSUM") as pw,
        ):
            # ---- constants ----
            # persistent logits slab for expmerge: two 1152-col halves,
            # alternated globally; each half holds a PAIR of subtiles so
            # exp runs once per pair (halves ACT per-op overhead)
            if CFG["expmerge"]:
                wpbig = pw.tile([128, 2304], F32, tag="wp")
            else:
                wpbig = None
            gpar = [0]  # global pair parity
            idxt = cpool.tile([128, nt * 8], I16, tag="idxt")
            nc.sync.dma_start(idxt[:], idx_d[:])
            cp = cpool.tile([128, 1473], BF16, tag="cpack")
            nc.sync.dma_start(cp[:], cpack_d[:])
            w3km0 = cp[:, 0:576]
            w3km1 = cp[:, 576:1152]
            w2t = cp[0:65, 1152:1408]
            w1t = cp[0:4, 1408:1473]

            xwin = x_windows_ap()

            GDEPTH = 2
            PDEPTH = 2
            gtiles = {}
            ptiles = {}

            def issue_pose(t):
                J = Js[t]
                _lbl(f'pose[{t}]')
                p3 = ppool.tile([4, JMAXT * 128], BF16, tag="p3")
                nc.sync.dma_start(
                    p3[:, 0:J * 128],
                    pose_d[:, base[t] * 128:(base[t] + J) * 128])
                ptiles[t] = p3

            def issue_gather(t):
                _lbl(f'gather[{t}]')
                g = gpool.tile([128, 1, 1152], BF16, tag="g")
                nc.gpsimd.dma_gather(
                    out_ap=g[:],
                    in_ap=xwin,
                    idxs_ap=idxt[:, t * 8:(t + 1) * 8],
                    num_idxs=128,
                    num_idxs_reg=128,
                    elem_size=1152,
                    elem_step=384,
                )
                gtiles[t] = g

            for t in range(min(PDEPTH, nt)):
                issue_pose(t)
            for t in range(min(GDEPTH, nt)):
                issue_gather(t)

            mlp_done = {}  # chunk idx -> (h1s, h2s)

            def mlp_thunks(i):
                """Per-q-chunk MLP thunks for chunk i (h1mm, h1relu, h2mms,
                h2relu); interleaved into the previous chunk's stage-1
                stream so evictions are ready early."""
                t, jb, jc, _ = chunks[i]
                p3 = ptiles[t]
                npx = jc * 128
                h1s = mpool.tile([65, JCAP * 128], BF16, tag="h1s")
                h2s = mpool.tile([128, 2, JCAP * 128], BF16, tag="h2s")
                mlp_done[i] = (h1s, h2s)

                def mk(q0):
                    def go():
                        qn = min(512, npx - q0)
                        qs = slice(jb * 128 + q0, jb * 128 + q0 + qn)
                        qd = slice(q0, q0 + qn)
                        _lbl(f'mlp[{i}]q{q0//512}')
                        h1p = ph1.tile([65, 512], F32, tag="h1p")
                        nc.tensor.matmul(h1p[:, 0:qn], w1t, p3[:, qs],
                                         start=True, stop=True)
                        nc.scalar.activation(h1s[:, qd], h1p[:, 0:qn],
                                             AF.Relu)
                        h2p = ph2.tile([128, 2, 512], F32, tag="h2p")
                        for cc in range(2):
                            nc.tensor.matmul(h2p[:, cc, 0:qn],
                                             w2t[:, cc * 128:(cc + 1) * 128],
                                             h1s[:, qd], start=True,
                                             stop=True)
                        m = CFG["h2relu"]
                        on_act = m == "act" or (m == "alt" and
                                                (q0 // 512) % 2 == 1)
                        if on_act:
                            nc.scalar.activation(h2s[:, :, qd],
                                                 h2p[:, :, 0:qn], AF.Relu)
                        else:
                            nc.gpsimd.tensor_scalar_max(h2s[:, :, qd],
                                                        h2p[:, :, 0:qn], 0.0)
                    return go

                return [mk(q0) for q0 in range(0, npx, 512)]

            stage2 = []

            def stage1_thunks(i):
                """Thunks for chunk i: per-subtile logits+exp, mul groups
                after every 4 exps, then d1/d2.  Appends to stage2."""
                t, jb, jc, slot0 = chunks[i]
                h1s, h2s = mlp_done.pop(i)
                g = gtiles[t]
                e_f = epool.tile([128, JCAP, 576], BF16, tag="e_t")
                e = e_f[:, 0:jc, :]
                g3 = g.rearrange("p o (k b c) -> p o k b c", k=9, b=2)
                prods = vpool.tile([128, JCAP, 9, 128], BF16, tag="prods")
                e5f = e.rearrange("p j (k c) -> p k j c", k=9)
                prvf = prods[:, 0:jc, :, :].rearrange(
                    "p j k (b c) -> p k j b c", b=2)

                def mk_logit_plain(s):
                    def go():
                        _lbl(f'logit[{i}]s{s}')
                        ss = slice(s * 128, s * 128 + 128)
                        wp = pw.tile([128, 576], F32, tag="wp")
                        for r0, r1 in ((0, 512), (512, 576)):
                            nc.tensor.matmul(wp[:, r0:r1], h2s[:, 0, ss],
                                             w3km0[:, r0:r1], start=True,
                                             stop=False)
                            nc.tensor.matmul(wp[:, r0:r1], h2s[:, 1, ss],
                                             w3km1[:, r0:r1], start=False,
                                             stop=True)
                        nc.scalar.activation(e[:, s, :], wp[:], AF.Exp)
                    return go

                def mk_logit_merged(s, half):
                    def go():
                        _lbl(f'logit[{i}]s{s}')
                        ss = slice(s * 128, s * 128 + 128)
                        c0 = half * 1152 + (s % 2) * 576
                        rsp = (c0 // 512 + 1) * 512
                        for r0, r1 in ((c0, rsp), (rsp, c0 + 576)):
                            nc.tensor.matmul(wpbig[:, r0:r1], h2s[:, 0, ss],
                                             w3km0[:, r0 - c0:r1 - c0],
                                             start=True, stop=False)
                            nc.tensor.matmul(wpbig[:, r0:r1], h2s[:, 1, ss],
                                             w3km1[:, r0 - c0:r1 - c0],
                                             start=False, stop=True)
                        if s % 2 == 1 or s + 1 == jc:
                            s0 = s - (s % 2)
                            nn = (s - s0 + 1) * 576
                            p0 = half * 1152
                            nc.scalar.activation(e[:, s0:s + 1, :],
                                                 wpbig[:, p0:p0 + nn], AF.Exp)
                    return go

                def mk_logit(s):
                    if not CFG["expmerge"]:
                        return mk_logit_plain(s)
                    if s % 2 == 0:
                        gpar[0] ^= 1
                    return mk_logit_merged(s, gpar[0])

                def mk_muls(j0, j1):
                    def go():
                        _lbl(f'mul[{i}]j{j0}')
                        pj0 = max(j0, min(CFG["mgrp"], j1))
                        pj1 = max(j0, min(CFG["mgrp"] + CFG["poolj"], j1))
                        for b in range(2):
                            gbf = g3[:, :, :, b, :].rearrange(
                                "p o k c -> p k (o c)").unsqueeze(2)
                            for a0, a1, eng in ((j0, pj0, nc.vector),
                                                (pj0, pj1, nc.gpsimd),
                                                (pj1, j1, nc.vector)):
                                if a0 >= a1:
                                    continue
                                gb = gbf.broadcast_to((128, 9, a1 - a0, 64))
                                eng.tensor_mul(prvf[:, :, a0:a1, b, :],
                                               gb, e5f[:, :, a0:a1, :])
                    return go

                def mk_d12(j0, j1):
                    def go():
                        _lbl(f'd12[{i}]')
                        eg = e[:, j0:j1, :]
                        nc.vector.tensor_add(eg[:, :, 0:256], eg[:, :, 0:256],
                                             eg[:, :, 256:512])
                        if CFG["d2"] == "dve":
                            nc.vector.tensor_add(eg[:, :, 0:128],
                                                 eg[:, :, 0:128],
                                                 eg[:, :, 128:256])
                    return go

                rec = (i, e, prods, jc, slot0)

                def mk_d1dma():
                    def go():
                        emit_d1dma(rec)
                    return go

                thunks = []
                j0 = 0
                for s in range(jc):
                    thunks.append(mk_logit(s))
                    if s + 1 == jc or (s + 1) % CFG['mgrp'] == 0:
                        thunks.append(mk_muls(j0, s + 1))
                        if CFG["d1"] == "dve":
                            thunks.append(mk_d12(j0, s + 1))
                        elif s + 1 == jc:
                            thunks.append(mk_d1dma())
                        thunks.append(None)  # slot for an MLP q-thunk
                        j0 = s + 1
                stage2.append(rec)
                return thunks

            def emit_d1dma(rec):
                # den level 1 on accumulating DMA: e[:,:,0:256] += e[:,:,256:512]
                i, e, prods, jc, slot0 = rec
                _lbl(f'd1dma[{i}]')
                nc.gpsimd.dma_start(e[:, :, 0:256], e[:, :, 256:512],
                                    accum_op=ADD)

            def emit_d2_dve(rec):
                i, e, prods, jc, slot0 = rec
                _lbl(f'd2[{i}]')
                nc.vector.tensor_add(e[:, :, 0:128], e[:, :, 0:128],
                                     e[:, :, 128:256])

            def emit_l2(rec):
                i, e, prods, jc, slot0 = rec
                _lbl(f'L2[{i}]')
                def ksl(a, b):
                    return prods[:, 0:jc, a:b, :].rearrange(
                        "p j k c -> p j (k c)")
                nc.gpsimd.dma_start(ksl(0, 2), ksl(2, 4), accum_op=ADD)
                if CFG["l3"] == "dma":
                    nc.gpsimd.dma_start(ksl(0, 1), ksl(1, 2), accum_op=ADD)

            def emit_l1(rec):
                i, e, prods, jc, slot0 = rec
                _lbl(f'L1[{i}]')
                def ksl(a, b):
                    return prods[:, 0:jc, a:b, :].rearrange(
                        "p j k c -> p j (k c)")
                nc.gpsimd.dma_start(ksl(0, 4), ksl(4, 8), accum_op=ADD)

            dens = {}

            def emit_d3fin(rec):
                # writes den to its own tile so e's buffer frees here (early
                # in the iteration) instead of at DVE's div at iteration end
                i, e, prods, jc, slot0 = rec
                _lbl(f'd3fin[{i}]')
                den_f = dpool.tile([128, JCAP, 64], BF16, tag="den")
                den = den_f[:, 0:jc, :]
                eng = nc.vector if CFG["d3fin"] == "dve" else nc.gpsimd
                eng.tensor_add(den, e[:, :, 0:64], e[:, :, 64:128])
                eng.tensor_add(den, den, e[:, :, 512:576])
                dens[i] = den

            def emit_acc_div(rec):
                i, e, prods, jc, slot0 = rec
                _lbl(f'accdiv[{i}]')
                k0 = prods[:, 0:jc, 0, :]
                if CFG["l3"] == "dve":
                    nc.vector.tensor_add(k0, k0, prods[:, 0:jc, 1, :])
                nc.vector.tensor_add(k0, k0, prods[:, 0:jc, 8, :])
                out_f = opool.tile([128, JCAP, 128], BF16, tag="out_t")
                out_t = out_f[:, 0:jc, :]
                ov = out_t.rearrange("p j (b c) -> p j b c", b=2)
                av = k0.rearrange("p j (b c) -> p j b c", b=2)
                den = dens.pop(i)
                rden_f = dpool.tile([128, JCAP, 64], BF16, tag="rden")
                rden = rden_f[:, 0:jc, :]
                with nc.allow_low_precision(reason="softmax denom in bf16"):
                    nc.vector.reciprocal(rden, den)
                dv = rden.unsqueeze(2).broadcast_to((128, jc, 2, 64))
                nc.vector.tensor_mul(ov, av, dv)
                nc.sync.dma_start(
                    out_d[:, slot0:slot0 + jc].rearrange("p s c -> p (s c)"),
                    out_t.rearrange("p s c -> p (s c)"))

            # pipeline per iteration i:
            #   Pool: d3fin(i-1), h2relus(i+1) (interleaved), L2(i-1), L1(i)
            #   PE:   logits(i) with MLP(i+1) q-blocks in the exp-wait gaps
            #   ACT:  exps(i) + h1relus(i+1) interleaved
            #   DVE:  mul+d12 groups(i), acc2+div(i-1)
            for th0 in mlp_thunks(0):
                th0()
            for i in range(nch):
                t = chunks[i][0]
                tn = chunks[i + 1][0] if i + 1 < nch else None
                if tn is not None and tn != t:
                    if tn + PDEPTH - 1 < nt:
                        issue_pose(tn + PDEPTH - 1)
                    if tn + GDEPTH - 1 < nt:
                        issue_gather(tn + GDEPTH - 1)
                prev = stage2.pop(0) if stage2 else None
                if prev is not None:
                    emit_d3fin(prev)
                    emit_l2(prev)
                th = stage1_thunks(i)
                mth = list(mlp_thunks(i + 1)) if i + 1 < nch else []
                mi = 0
                for thunk in th:
                    if thunk is None:
                        if mi < len(mth):
                            mth[mi]()
                            mi += 1
                    else:
                        thunk()
                while mi < len(mth):
                    mth[mi]()
                    mi += 1
                if prev is not None:
                    emit_acc_div(prev)
                if CFG["d1"] == "dma2":
                    # d2 for the CURRENT chunk at DVE's iteration end: the
                    # d1 accum-DMA (fired right after the muls) is complete
                    # by the time DVE drains its finish ops
                    emit_d2_dve(stage2[-1])
                emit_l1(stage2[-1])
                if CFG["d2"] == "pool":
                    # after L1 in Pool's queue: d2's dep (d1 on DVE) resolves
                    # later than L1's (muls), so this order avoids head-block
                    ii, ee, _pr, _jc, _sl = stage2[-1]
                    _lbl(f'd2p[{ii}]')
                    nc.gpsimd.tensor_add(ee[:, :, 0:128], ee[:, :, 0:128],
                                         ee[:, :, 128:256])
            # drain last chunk
            rec = stage2.pop(0)
            emit_d3fin(rec)
            emit_l2(rec)
            emit_acc_div(rec)

    nc.compile()
    return nc


def _schedule(interMapY, interMapX):
    Y = np.asarray(interMapY).astype(np.int64).reshape(-1)
    X = np.asarray(interMapX).astype(np.int64).reshape(-1)
    m = (Y * SCW + X).astype(np.int32)
    order = np.argsort(m, kind='stable')
    ms = m[order]
    uniq, first, inv_s, counts = np.unique(
        ms, return_index=True, return_inverse=True, return_counts=True)
    ncell = len(uniq)
    # split cells with >JCAP pixels into <=JCAP-pixel pseudo-cells (each
    # segment re-gathers the same window; gathers are cheap).  All counts
    # then fit a single chunk, tiles get near-zero occupancy spread, and
    # ragged multi-chunk tiles disappear.
    nseg = (counts + JCAP - 1) // JCAP
    segbase = np.concatenate([[0], np.cumsum(nseg)])[:-1]
    P = int(nseg.sum())
    pc_cell = np.repeat(np.arange(ncell), nseg)
    seg_idx = np.arange(P) - np.repeat(segbase, nseg)
    # balanced segment sizes (~n/nseg each, all in 9..16 for split cells):
    # avoids tiny residual segments that would form tiny-J tail tiles
    nn, kk = counts[pc_cell], nseg[pc_cell]
    counts0 = counts
    counts = nn // kk + (seg_idx < nn % kk)
    uniq = uniq[pc_cell]
    # per-pixel segment index and within-segment slot under balanced sizes
    off_px = np.arange(PXTOT) - first[inv_s]
    pN = counts0[inv_s]
    pK = (pN + JCAP - 1) // JCAP
    lo = pN // pK
    hi = pN % pK
    cut = hi * (lo + 1)
    seg_px = np.where(off_px < cut, off_px // (lo + 1),
                      hi + (off_px - cut) // np.maximum(lo, 1))
    j_px = np.where(off_px < cut, off_px % (lo + 1),
                    (off_px - cut) % np.maximum(lo, 1))
    pp_px = segbase[inv_s] + seg_px
    ncell = P
    C8 = ((ncell + 1023) // 1024) * 1024
    padn = C8 - ncell
    counts_p = np.concatenate([counts, np.zeros(padn, counts.dtype)])
    uniq_p = np.concatenate([uniq, np.zeros(padn, uniq.dtype)])
    rank_order = np.argsort(-counts_p, kind='stable')  # cellidx by rank
    R = C8
    ranks = np.arange(R)
    blk, pos = ranks // 8, ranks % 8
    core_of_rank = np.where(blk % 2 == 0, pos, 7 - pos)
    percore_pos = np.zeros(R, np.int64)
    for c in range(NCORES):
        sel = core_of_rank == c
        percore_pos[sel] = np.arange(sel.sum())
    nt = (C8 // NCORES) // 128
    # pyramid tile order: small-J tiles at both ends (short pipeline fill
    # AND short drain), largest in the middle
    blk = percore_pos // 128
    asc = np.arange(nt - 1, -1, -1)  # blocks by J ascending
    schd = np.concatenate([asc[0::2], asc[1::2][::-1]])
    pos_of_block = np.empty(nt, np.int64)
    pos_of_block[schd] = np.arange(nt)
    tile_of = pos_of_block[blk]
    part_of = percore_pos % 128
    cnt_r = counts_p[rank_order]
    Js = np.zeros(nt, np.int64)
    for t in range(nt):
        Js[t] = max(1, cnt_r[tile_of == t].max())
    base = np.concatenate([[0], np.cumsum(Js)])[:-1]
    ST = int(Js.sum())

    rank_of_cell = np.empty(R, np.int64)
    rank_of_cell[rank_order] = ranks
    r_px = rank_of_cell[pp_px]  # per sorted-pixel (pseudo-cell rank)
    core_px = core_of_rank[r_px]
    slot_px = base[tile_of[r_px]] + j_px
    part_px = part_of[r_px]

    win = np.zeros((NCORES, nt, 128), np.int64)
    win[core_of_rank, tile_of, part_of] = uniq_p[rank_order]
    return dict(order=order, core_px=core_px, slot_px=slot_px,
                part_px=part_px, Js=tuple(int(j) for j in Js), base=base,
                ST=ST, win=win, nt=nt)


def _host_prep(x, poseMap, W1, b1, W2, b2, W3, b3, sch):
    bf = ml_dtypes.bfloat16
    xp = np.pad(np.asarray(x, np.float32), ((0, 0), (0, 0), (1, 1), (1, 1)))
    xs = np.ascontiguousarray(np.transpose(xp, (2, 3, 0, 1)))
    sw = np.lib.stride_tricks.sliding_window_view(xs, 3, axis=0)
    xsc = np.ascontiguousarray(np.transpose(sw, (0, 1, 4, 2, 3)))
    xsc = xsc.reshape(NSC * 384).astype(bf)

    ST, nt = sch["ST"], sch["nt"]
    pose_full = np.concatenate(
        [np.asarray(poseMap, np.float32)[0].reshape(3, PXTOT),
         np.ones((1, PXTOT), np.float32)], axis=0)

    w1t32 = np.zeros((4, 65), np.float32)
    w1t32[0:3, 0:64] = np.asarray(W1, np.float32).T
    w1t32[3, 0:64] = np.asarray(b1, np.float32)
    w1t32[3, 64] = 1.0
    w2t32 = np.concatenate([np.asarray(W2, np.float32).T,
                            np.asarray(b2, np.float32)[None, :]], axis=0)
    W3r = np.asarray(W3, np.float32).reshape(C, KS, KS, 256)
    w3km32 = np.ascontiguousarray(
        np.transpose(W3r, (3, 2, 1, 0))).reshape(256, 576)
    # fold b3 into... b3 is zeros in this problem; assert to be safe
    b3a = np.asarray(b3, np.float32)

    cpack = np.zeros((128, 1473), np.float32)
    cpack[:, 0:576] = w3km32[0:128]
    cpack[:, 576:1152] = w3km32[128:256]
    cpack[0:65, 1152:1408] = w2t32
    cpack[0:4, 1408:1473] = w1t32
    cpack = cpack.astype(bf)

    in_maps = []
    for c in range(NCORES):
        sel = sch["core_px"] == c
        pose = np.zeros((4, ST * 128), np.float32)
        cols = sch["slot_px"][sel] * 128 + sch["part_px"][sel]
        pose[:, cols] = pose_full[:, sch["order"][sel]]
        ids = sch["win"][c]  # [nt, 128]
        idxw = np.zeros((128, nt * 8), np.int16)
        for t in range(nt):
            a = ids[t].astype(np.int16).reshape(8, 16)  # [cc, 16]
            idxw[:, t * 8:(t + 1) * 8] = np.tile(a.T, (8, 1))
        in_maps.append({
            "xw": xsc,
            "idx": np.ascontiguousarray(idxw),
            "pose": np.ascontiguousarray(pose).astype(bf),
            "cpack": cpack,
        })
    return in_maps, b3a


def kernel(**inputs):
    global LAST_RESULT, LAST_PROG
    sch = _schedule(inputs["interMapY"], inputs["interMapX"])
    key = sch["Js"]
    if key not in _PROG_CACHE:
        _PROG_CACHE[key] = build_program(sch["Js"])
    nc = _PROG_CACHE[key]
    LAST_PROG = nc
    in_maps, b3a = _host_prep(
        inputs["x"], inputs["poseMap"], inputs["W1"], inputs["b1"],
        inputs["W2"], inputs["b2"], inputs["W3"], inputs["b3"], sch)
    os.environ.setdefault("BASS_NEVER_TRACE", "1")
    res = None
    last_err = None
    for attempt in range(3):
        try:
            res = run_bass_kernel_spmd(nc, in_maps, list(range(NCORES)))
            break
        except Exception as err:
            last_err = err
            os.environ["NEURON_RT_RESET_CORES"] = "1"
    if res is None:
        raise last_err
    LAST_RESULT = res
    ST = sch["ST"]
    out_full = np.zeros((PXTOT, 128), np.float32)
    for c in range(NCORES):
        arr = np.asarray(res.results[c]["out"]).reshape(128, ST, 128)
        sel = sch["core_px"] == c
        out_full[sch["order"][sel]] = arr[
            sch["part_px"][sel], sch["slot_px"][sel], :].astype(np.float32)
    out = out_full.reshape(HO, WO, BS, C).transpose(2, 3, 0, 1)
    # b3 correction: b3 is zero in this problem's setup; softmax with b3
    # would change weights -- recompute would be needed.  Guard loudly.
    if np.any(b3a != 0.0):
        raise NotImplementedError("nonzero b3 not folded in this kernel")
    return np.ascontiguousarray(out)


if __name__ == "__main__":
    data = np.load(sys.argv[1] if len(sys.argv) > 1 else "work/inputs.npz")
    out = kernel(**{k: data[k] for k in data.files})
    print("out", out.shape, out.dtype, float(np.abs(out).max()))
